# revision 16
# baseline (speedup 1.0000x reference)
"""Trainium2 Bass kernel for LocalDenseSynthesizerAttention.

Data-parallel over batch B=8 -> 8 cores, one batch each. Wire-traffic and
dispatch optimized for the axon tunnel (~90MB/s each way, full duplex):
  - jitted executables built once and cached (no per-call retrace)
  - q shipped t-major fp8 (e4m3) and transposed on-device (PE transpose);
    v shipped t-major bf16 and transposed on-device via XBAR DMA;
    output returned bf16 and widened exactly on host
  - w1/w2 shipped fp8 scaled x16 (rescaled on device via activation scale),
    w3/w_out bf16; shipped as 8-way shards once per call to a tiny
    weights launch that AllGathers them on device; the full per-core
    weights stay device-resident and feed the compute launches
  - compute is split into sequence chunks (the attention window is local,
    halo = 22), one 8-core launch per chunk: chunk i+1's upload overlaps
    chunk i's exec + download
  - donated output buffers created on-device (no zeros upload)

The local window C=45 weighted sum is computed as banded matmuls: the banded
matrix B[s,t'] = attn[t0+t',h,s-t'] is an affine strided view of a zero-padded
attn tensor in DRAM, loaded matmul-ready via XBAR transpose-DMA.

Self-contained: hardcodes shapes from the problem spec.
"""
import sys
sys.path.insert(0, '/opt/trn_rl_repo')
import numpy as np
import ml_dtypes

import concourse.bass as bass
import concourse.mybir as mybir
import concourse.tile as tile
from concourse import bacc
from concourse import masks

T, F = 2048, 512
H, C, DK = 8, 45, 64
HC = H * C          # 360
W = 128             # padded attn width per head (covers s-t' in [-63,127])
S = 64              # t' band-block size
PADV = 22           # (C-1)//2
KF = F // 128       # 4 contraction chunks
B = 8               # total batches / cores
FSH = F // B        # 64 weight-shard rows per core

NCHUNK = 4
TC = T // NCHUNK    # sequence-chunk length
VH = 32             # v halo rows each side (>= PADV; host rows stay %16)
TVH = TC + 2 * VH   # host-supplied v rows per chunk (576)
TV = ((TVH + 127) // 128) * 128   # device-padded to full 128-row tiles (640)
VOFF = VH - PADV    # chunk-vpad[r] = v_in[r + VOFF]

BF16 = mybir.dt.bfloat16
FP8 = mybir.dt.float8e4
F32 = mybir.dt.float32
WSCALE = 16.0       # fp8 weight pre-scale for w1/w2

_CACHE = {}


def _build_w():
    """Tiny weights launch: AllGather 8-way weight shards into full
    per-core weights (device-resident outputs)."""
    nc = bacc.Bacc("TRN2", target_bir_lowering=False, debug=False,
                   num_devices=B)
    w1s = nc.dram_tensor("w1s", (FSH, F), FP8, kind="ExternalInput")
    w2s = nc.dram_tensor("w2s", (FSH, HC), FP8, kind="ExternalInput")
    w3s = nc.dram_tensor("w3s", (FSH, F), BF16, kind="ExternalInput")
    wos = nc.dram_tensor("wos", (FSH, F), BF16, kind="ExternalInput")
    w1f = nc.dram_tensor("w1f", (F, F), FP8, kind="ExternalOutput")
    w2f = nc.dram_tensor("w2f", (F, HC), FP8, kind="ExternalOutput")
    w3f = nc.dram_tensor("w3f", (F, F), BF16, kind="ExternalOutput")
    wof = nc.dram_tensor("wof", (F, F), BF16, kind="ExternalOutput")
    groups = [list(range(B))]
    with tile.TileContext(nc) as tc:
        with tc.tile_pool(name="dram", bufs=1, space="DRAM") as dp:
            # collectives cannot read IO tensors: stage shards first
            stages = (dp.tile([FSH, F], FP8, name="st1"),
                      dp.tile([FSH, HC], FP8, name="st2"),
                      dp.tile([FSH, F], BF16, name="st3"),
                      dp.tile([FSH, F], BF16, name="st4"))
            fulls = (dp.tile([F, F], FP8, name="fu1"),
                     dp.tile([F, HC], FP8, name="fu2"),
                     dp.tile([F, F], BF16, name="fu3"),
                     dp.tile([F, F], BF16, name="fu4"))
            for shard, stage, full, out in zip(
                    (w1s, w2s, w3s, wos), stages, fulls,
                    (w1f, w2f, w3f, wof)):
                nc.sync.dma_start(stage[:, :], shard[:, :])
                nc.gpsimd.collective_compute(
                    "AllGather", mybir.AluOpType.bypass, groups,
                    [stage[:, :]], [full[:, :]])
                nc.sync.dma_start(out[:, :], full[:, :])
    nc.compile()
    return nc


def _build_k():
    """Compute launch for one sequence chunk of TC rows."""
    NT128 = TC // 128           # t-tiles in the chunk
    NTV = TV // 128             # t-tiles of the v input (incl halo)
    NB = TC // S                # band blocks
    nc = bacc.Bacc("TRN2", target_bir_lowering=False, debug=False,
                   num_devices=B)
    q = nc.dram_tensor("q", (TC, F), FP8, kind="ExternalInput")
    v = nc.dram_tensor("v", (TVH, F), BF16, kind="ExternalInput")
    w1f = nc.dram_tensor("w1f", (F, F), FP8, kind="ExternalInput")
    w2f = nc.dram_tensor("w2f", (F, HC), FP8, kind="ExternalInput")
    w3f = nc.dram_tensor("w3f", (F, F), BF16, kind="ExternalInput")
    wof = nc.dram_tensor("wof", (F, F), BF16, kind="ExternalInput")
    out = nc.dram_tensor("out", (TC, F), BF16, kind="ExternalOutput")

    with tile.TileContext(nc) as tc:
        with tc.tile_pool(name="wpool", bufs=1) as wp, \
             tc.tile_pool(name="inpool", bufs=1) as inp, \
             tc.tile_pool(name="persist", bufs=1) as pers, \
             tc.tile_pool(name="work", bufs=2) as wk, \
             tc.tile_pool(name="band", bufs=4) as bp, \
             tc.tile_pool(name="psmain", bufs=2, space="PSUM") as psm, \
             tc.tile_pool(name="psband", bufs=4, space="PSUM") as psb, \
             tc.tile_pool(name="pstp", bufs=2, space="PSUM") as ptp, \
             tc.tile_pool(name="drampool", bufs=1, space="DRAM") as dp:

            # ---- weights to SBUF, [128, KF, n] layout (partition = contraction)
            w1_t = wp.tile([128, KF, F], FP8, tag="w1")
            nc.sync.dma_start(w1_t[:], w1f[:, :].rearrange("(ko p) n -> p ko n", p=128))
            w2_t = wp.tile([128, KF, HC], FP8, tag="w2")
            nc.sync.dma_start(w2_t[:], w2f[:, :].rearrange("(ko p) n -> p ko n", p=128))
            w3_t = wp.tile([128, KF, F], BF16, tag="w3")
            nc.sync.dma_start(w3_t[:], w3f[:, :].rearrange("(ko p) n -> p ko n", p=128))
            wo_t = wp.tile([128, KF, F], BF16, tag="wo")
            nc.sync.dma_start(wo_t[:], wof[:, :].rearrange("(ko p) n -> p ko n", p=128))

            # ---- v (t-major bf16): XBAR transpose to f-major; cols beyond
            # the host-supplied TVH rows are zeroed so the padded v-proj
            # tiles project to exact zeros
            vT_t = inp.tile([128, KF, TV], BF16, tag="vT")
            if TV > TVH:
                nc.any.memzero(vT_t[:, :, TVH:TV])
            for fo in range(KF):
                eng = nc.scalar if fo % 2 else nc.sync
                eng.dma_start_transpose(vT_t[:, fo, 0:TVH],
                                        v[:, fo * 128:(fo + 1) * 128])

            # ---- q (t-major fp8): PE-transpose to f-major
            ident = pers.tile([128, 128], FP8, tag="ident")
            masks.make_identity(nc, ident[:])
            qT_t = inp.tile([128, KF, TC], FP8, tag="qT")
            for tt in range(NT128):
                qstage = wk.tile([128, F], FP8, tag="qstage")
                nc.sync.dma_start(qstage[:], q[tt * 128:(tt + 1) * 128, :])
                for fo in range(KF):
                    # fp8 PE transpose requires output element step of 2
                    pst = ptp.tile([128, 256], FP8, tag="qtp")
                    pstv = pst[:].rearrange("p (a b) -> p a b", b=2)[:, :, 0]
                    nc.tensor.transpose(pstv,
                                        qstage[:, fo * 128:(fo + 1) * 128],
                                        ident[:])
                    nc.scalar.copy(qT_t[:, fo, tt * 128:(tt + 1) * 128],
                                   pstv)

            # ---- DRAM scratch
            # vproj rows j = w3-projection of v_in row j; chunk-vpad[r] = row
            # r + VOFF; v_in's zero halo rows project to exact zeros
            vproj = dp.tile([TV, F], BF16)
            # apad: 1 guard row + TC data rows + 1 guard row, row = [8 heads x 128]
            apad = dp.tile([TC + 2, H * W], BF16)

            # zero tile for apad guards
            z_t = pers.tile([128, H * W], BF16, tag="zt")
            nc.any.memzero(z_t[:])
            nc.sync.dma_start(apad[0:1, :], z_t[0:1, :])
            nc.sync.dma_start(apad[TC + 1:TC + 2, :], z_t[0:1, :])

            # ---- persistent SBUF activations
            qrT = pers.tile([128, KF, TC], FP8, tag="qrT")   # relu(q @ w1), f-major
            xT = pers.tile([128, KF, TC], BF16, tag="xT")    # band output, f-major

            # ================= Phase A: q-proj + relu (f-major out) ===========
            # PSUM = q @ (16 w1); Relu(psum/16) -> fp8
            for fo in range(KF):
                for tt in range(TC // 512):
                    ps = psm.tile([128, 512], F32, tag="mm")
                    for k in range(KF):
                        nc.tensor.matmul(
                            ps[:], w1_t[:, k, fo * 128:(fo + 1) * 128],
                            qT_t[:, k, tt * 512:(tt + 1) * 512],
                            start=(k == 0), stop=(k == KF - 1))
                    nc.scalar.activation(qrT[:, fo, tt * 512:(tt + 1) * 512], ps[:],
                                         mybir.ActivationFunctionType.Relu,
                                         scale=1.0 / WSCALE)

            # ================= Phase C: v-proj (t-major out) -> vproj =========
            for tb in range(NTV):
                ps = psm.tile([128, 512], F32, tag="mm")
                for k in range(KF):
                    nc.tensor.matmul(
                        ps[:], vT_t[:, k, tb * 128:(tb + 1) * 128],
                        w3_t[:, k, :],
                        start=(k == 0), stop=(k == KF - 1))
                v_sb = wk.tile([128, F], BF16, tag="vsb")
                nc.scalar.copy(v_sb[:], ps[:])
                nc.sync.dma_start(vproj[tb * 128:(tb + 1) * 128, :], v_sb[:])

            # ====== Phase B: s-proj (t-major) + softmax -> apad (padded) ======
            # PSUM = qr @ (16 w2); Exp(psum/16)
            for tb in range(NT128):
                ps = psm.tile([128, 512], F32, tag="mm")
                for k in range(KF):
                    nc.tensor.matmul(
                        ps[:, 0:HC], qrT[:, k, tb * 128:(tb + 1) * 128],
                        w2_t[:, k, :],
                        start=(k == 0), stop=(k == KF - 1))
                e_t = wk.tile([128, HC], F32, tag="et")
                nc.scalar.activation(e_t[:], ps[:, 0:HC],
                                     mybir.ActivationFunctionType.Exp,
                                     scale=1.0 / WSCALE)
                zs = wk.tile([128, H], F32, tag="zs")
                nc.vector.reduce_sum(zs[:], e_t[:].rearrange("p (h c) -> p h c", c=C),
                                     axis=mybir.AxisListType.X)
                rz = wk.tile([128, H], F32, tag="rz")
                nc.vector.reciprocal(rz[:], zs[:])
                ap_t = wk.tile([128, H * W], BF16, tag="apad")
                if tb < 2:
                    # zero the pad region once per pool slot (bufs=2); the pad
                    # columns are never overwritten afterwards
                    nc.any.memzero(ap_t[:])
                nc.vector.tensor_mul(
                    out=ap_t[:].rearrange("p (h w) -> p h w", w=W)[:, :, 0:C],
                    in0=e_t[:].rearrange("p (h c) -> p h c", c=C),
                    in1=rz[:, :, None].to_broadcast((128, H, C)))
                nc.sync.dma_start(apad[1 + tb * 128:1 + (tb + 1) * 128, :], ap_t[:])

            # ================= Phase D: banded attention matmuls ==============
            # x[t', h*64+d] = sum_s chunkvpad[t0+s, h*64+d] * B_h[s, t']
            # B_h loaded via transpose-DMA of sheared apad view.
            apad_h = apad.tensor  # underlying DRAM handle
            apad_off = apad.offset if isinstance(apad.offset, int) else 0
            for g in range(NB // 4):    # groups of 4 band blocks = 256 t'
                pss = [psb.tile([128, 512], F32, tag="px", name=f"px{g}_{pi}")
                       for pi in range(4)]
                for j in range(4):
                    bi = g * 4 + j
                    t0 = S * bi
                    vsp = wk.tile([128, F], BF16, tag="vsp")
                    nc.sync.dma_start(vsp[:], vproj[VOFF + t0:VOFF + t0 + 128, :])
                    for p in range(4):      # head pairs
                        for i in range(2):
                            h = 2 * p + i
                            b_t = bp.tile([W, S], BF16, tag="bt")
                            src = bass.AP(
                                tensor=apad_h,
                                offset=apad_off + (1 + t0) * (H * W) + h * W,
                                ap=[[H * W - 1, S], [1, W]])
                            eng = nc.scalar if h % 2 else nc.sync
                            eng.dma_start_transpose(b_t[:], src)
                            # lhsT = v head-pair [128, 128]; valid out rows are
                            # [i*64:(i+1)*64]; the other half is garbage and
                            # ignored at copyback.
                            nc.tensor.matmul(
                                pss[p][:, j * 128 + i * 64: j * 128 + (i + 1) * 64],
                                vsp[:, p * 128:(p + 1) * 128], b_t[:],
                                start=True, stop=True)
                # copy valid quadrants -> xT (f-major): fold p rows 0:63 = head
                # 2p (cols i=0), rows 64:127 = head 2p+1 (cols i=1)
                for p in range(4):
                    ps3 = pss[p][:].rearrange("d (j i k) -> d j i k", j=4, i=2)
                    dst = xT[:, p, g * 256:(g + 1) * 256] \
                        .rearrange("d (j k) -> d j k", j=4)
                    nc.vector.tensor_copy(out=dst[0:64], in_=ps3[0:64, :, 0, :])
                    nc.vector.tensor_copy(out=dst[64:128], in_=ps3[64:128, :, 1, :])

            # ================= Phase E: out-proj (t-major out) ================
            for tb in range(NT128):
                ps = psm.tile([128, 512], F32, tag="mm")
                for k in range(KF):
                    nc.tensor.matmul(
                        ps[:], xT[:, k, tb * 128:(tb + 1) * 128],
                        wo_t[:, k, :],
                        start=(k == 0), stop=(k == KF - 1))
                o_sb = wk.tile([128, F], BF16, tag="osb")
                nc.scalar.copy(o_sb[:], ps[:])
                nc.sync.dma_start(out[tb * 128:(tb + 1) * 128, :], o_sb[:])

    nc.compile()
    return nc


def _make_exec(nc, devices):
    """Cached jitted executable + on-device zeros maker for one bass module."""
    import jax
    import jax.numpy as jnp
    from jax.sharding import Mesh, PartitionSpec, NamedSharding
    from jax.experimental.shard_map import shard_map
    from concourse.bass2jax import _bass_exec_p, partition_id_tensor

    partition_name = (nc.partition_id_tensor.name
                      if nc.partition_id_tensor else None)
    in_names, out_names, out_avals = [], [], []
    for alloc in nc.m.functions[0].allocations:
        if not isinstance(alloc, mybir.MemoryLocationSet):
            continue
        if alloc.kind not in ("ExternalInput", "ExternalOutput"):
            continue
        name = alloc.memorylocations[0].name
        if alloc.kind == "ExternalInput":
            if name != partition_name:
                in_names.append(name)
        else:
            out_avals.append(jax.core.ShapedArray(
                tuple(alloc.tensor_shape), mybir.dt.np(alloc.dtype)))
            out_names.append(name)
    n_params, n_outs = len(in_names), len(out_avals)
    in_names_all = list(in_names) + list(out_names)
    if partition_name is not None:
        in_names_all.append(partition_name)

    def _body(*args):
        operands = list(args)
        if partition_name is not None:
            operands.append(partition_id_tensor())
        return tuple(_bass_exec_p.bind(
            *operands,
            out_avals=tuple(out_avals),
            in_names=tuple(in_names_all),
            out_names=tuple(out_names),
            lowering_input_output_aliases=(),
            sim_require_finite=True,
            sim_require_nnan=True,
            nc=nc))

    n = len(devices)
    mesh = Mesh(np.asarray(devices), ("core",))
    in_specs = (PartitionSpec("core"),) * (n_params + n_outs)
    out_specs = (PartitionSpec("core"),) * n_outs
    donate = tuple(range(n_params, n_params + n_outs))
    sharded = jax.jit(
        shard_map(_body, mesh=mesh, in_specs=in_specs, out_specs=out_specs,
                  check_rep=False),
        donate_argnums=donate, keep_unused=True)
    shard = NamedSharding(mesh, PartitionSpec("core"))
    mkzeros = jax.jit(
        lambda: tuple(jnp.zeros((n * a.shape[0], *a.shape[1:]), a.dtype)
                      for a in out_avals),
        out_shardings=(shard,) * n_outs)
    return {"sharded": sharded, "mkzeros": mkzeros, "in_names": in_names,
            "out_names": out_names, "shard": shard, "n": n}


def _get_state():
    if "state" in _CACHE:
        return _CACHE["state"]
    import jax
    from concourse.bass2jax import install_neuronx_cc_hook
    install_neuronx_cc_hook()
    devices = jax.devices()[:B]
    wexec = _make_exec(_build_w(), devices)
    kexec = _make_exec(_build_k(), devices)
    state = {"w": wexec, "k": kexec}
    _CACHE["state"] = state
    return state


def _to_bf16_bits(x32):
    """fp32 -> bf16 via round-half-up on the upper 16 bits (RNE-grade error,
    much faster than ml_dtypes astype). Returns uint16 bit pattern."""
    v = np.ascontiguousarray(x32).view(np.uint32)
    return np.right_shift(v + np.uint32(0x8000), 16).astype(np.uint16)


def kernel(query, key, value, w1, w2, w3, w_out, _trace=False):
    import jax
    st = _get_state()
    e4 = ml_dtypes.float8_e4m3
    bf = ml_dtypes.bfloat16
    wx, kx = st["w"], st["k"]

    query = np.asarray(query)
    value = np.asarray(value)

    # ---- weights launch first: tiny upload, runs while v/q upload
    wzeros = wx["mkzeros"]()
    warrs = {"w1s": (np.asarray(w1) * WSCALE).astype(e4),
             "w2s": (np.asarray(w2) * WSCALE).astype(e4),
             "w3s": _to_bf16_bits(np.asarray(w3)).view(bf),
             "wos": _to_bf16_bits(np.asarray(w_out)).view(bf)}
    wouts = wx["sharded"](*[warrs[n] for n in wx["in_names"]], *wzeros)
    wfull = dict(zip(wx["out_names"], wouts))

    # ---- per-chunk compute launches, pipelined
    pending = []
    for ci in range(NCHUNK):
        c0 = ci * TC
        kzeros = kx["mkzeros"]()
        # v chunk with halo, zero-padded at sequence edges
        vbuf = np.zeros((B, TVH, F), np.uint16)
        lo, hi = max(0, c0 - VH), min(T, c0 + TC + VH)
        off = lo - (c0 - VH)
        vbuf[:, off:off + (hi - lo)] = _to_bf16_bits(value[:, lo:hi])
        v_dev = jax.device_put(vbuf.view(bf).reshape(B * TVH, F), kx["shard"])
        # q chunk, t-major fp8 (transposed on device)
        q8 = query[:, c0:c0 + TC, :].astype(e4).reshape(B * TC, F)
        q_dev = jax.device_put(q8, kx["shard"])
        arrays = {"q": q_dev, "v": v_dev, "w1f": wfull["w1f"],
                  "w2f": wfull["w2f"], "w3f": wfull["w3f"],
                  "wof": wfull["wof"]}
        ins = [arrays[n] for n in kx["in_names"]]
        pending.append(kx["sharded"](*ins, *kzeros))

    # ---- collect: widen bf16 -> fp32 exactly (zero-extension)
    buf = np.zeros((B, T, F, 2), np.uint16)
    for ci, outs in enumerate(pending):
        c0 = ci * TC
        o16 = np.asarray(outs[0]).view(np.uint16).reshape(B, TC, F)
        buf[:, c0:c0 + TC, :, 1] = o16
    return buf.view(np.float32)[..., 0]


# revision 23
# speedup vs baseline: 1.1073x; 1.1073x over previous
"""Trainium2 Bass kernel for LocalDenseSynthesizerAttention.

Data-parallel over batch B=8 -> 8 cores, one batch each. Wire-traffic and
dispatch optimized for the axon tunnel (~90MB/s each way, full duplex):
  - jitted executables built once and cached (no per-call retrace)
  - q shipped t-major fp8 (e4m3) and transposed on-device (PE transpose);
    v shipped t-major bf16 and transposed on-device via XBAR DMA;
    output returned bf16 and widened exactly on host
  - w1/w2 shipped fp8 scaled x16 (rescaled on device via activation scale),
    w3/w_out bf16; shipped as 8-way shards once per call to a tiny
    weights launch that AllGathers them on device; the full per-core
    weights stay device-resident and feed the compute launches
  - compute is split into sequence chunks (the attention window is local,
    halo = 22), one 8-core launch per chunk: chunk i+1's upload overlaps
    chunk i's exec + download
  - donated output buffers created on-device (no zeros upload)

The local window C=45 weighted sum is computed as banded matmuls: the banded
matrix B[s,t'] = attn[t0+t',h,s-t'] is an affine strided view of a zero-padded
attn tensor in DRAM, loaded matmul-ready via XBAR transpose-DMA.

Self-contained: hardcodes shapes from the problem spec.
"""
import sys
sys.path.insert(0, '/opt/trn_rl_repo')
import numpy as np
import ml_dtypes

import concourse.bass as bass
import concourse.mybir as mybir
import concourse.tile as tile
from concourse import bacc
from concourse import masks

T, F = 2048, 512
H, C, DK = 8, 45, 64
HC = H * C          # 360
W = 128             # padded attn width per head (covers s-t' in [-63,127])
S = 64              # t' band-block size
PADV = 22           # (C-1)//2
KF = F // 128       # 4 contraction chunks
B = 8               # total batches / cores
FSH = F // B        # 64 weight-shard rows per core

NCHUNK = 2
TC = T // NCHUNK    # sequence-chunk length
VH = 64             # v halo rows each side (>= PADV, keeps tiles 128-aligned)
TV = TC + 2 * VH    # logical v rows per chunk (main + halo)
VOFF = VH - PADV    # chunk-vpad[r] = v_logical[r + VOFF]

BF16 = mybir.dt.bfloat16
FP8 = mybir.dt.float8e4
F32 = mybir.dt.float32
WSCALE = 16.0       # fp8 weight pre-scale for w1/w2

_CACHE = {}


def _build_w():
    """Tiny weights launch: AllGather 8-way weight shards into full
    per-core weights (device-resident outputs)."""
    nc = bacc.Bacc("TRN2", target_bir_lowering=False, debug=False,
                   num_devices=B)
    w1s = nc.dram_tensor("w1s", (FSH, F), FP8, kind="ExternalInput")
    w2s = nc.dram_tensor("w2s", (FSH, HC), FP8, kind="ExternalInput")
    w3s = nc.dram_tensor("w3s", (FSH, F), BF16, kind="ExternalInput")
    wos = nc.dram_tensor("wos", (FSH, F), BF16, kind="ExternalInput")
    w1f = nc.dram_tensor("w1f", (F, F), FP8, kind="ExternalOutput")
    w2f = nc.dram_tensor("w2f", (F, HC), FP8, kind="ExternalOutput")
    w3f = nc.dram_tensor("w3f", (F, F), BF16, kind="ExternalOutput")
    wof = nc.dram_tensor("wof", (F, F), BF16, kind="ExternalOutput")
    groups = [list(range(B))]
    with tile.TileContext(nc) as tc:
        with tc.tile_pool(name="dram", bufs=1, space="DRAM") as dp:
            # collectives cannot read IO tensors: stage shards first
            stages = (dp.tile([FSH, F], FP8, name="st1"),
                      dp.tile([FSH, HC], FP8, name="st2"),
                      dp.tile([FSH, F], BF16, name="st3"),
                      dp.tile([FSH, F], BF16, name="st4"))
            fulls = (dp.tile([F, F], FP8, name="fu1"),
                     dp.tile([F, HC], FP8, name="fu2"),
                     dp.tile([F, F], BF16, name="fu3"),
                     dp.tile([F, F], BF16, name="fu4"))
            for shard, stage, full, out in zip(
                    (w1s, w2s, w3s, wos), stages, fulls,
                    (w1f, w2f, w3f, wof)):
                nc.sync.dma_start(stage[:, :], shard[:, :])
                nc.gpsimd.collective_compute(
                    "AllGather", mybir.AluOpType.bypass, groups,
                    [stage[:, :]], [full[:, :]])
                nc.sync.dma_start(out[:, :], full[:, :])
    nc.compile()
    return nc


def _build_k():
    """Compute launch for one sequence chunk of TC rows."""
    NT128 = TC // 128           # t-tiles in the chunk
    NTV = TV // 128             # t-tiles of the v input (incl halo)
    NB = TC // S                # band blocks
    nc = bacc.Bacc("TRN2", target_bir_lowering=False, debug=False,
                   num_devices=B)
    q = nc.dram_tensor("q", (TC, F), FP8, kind="ExternalInput")
    # v is split so the main part exactly matches the output shape/dtype and
    # can be donated/aliased as the output buffer (saves a zeros launch):
    # vh rows [0, VH) = rows just before the chunk, [VH, 2VH) = just after
    v = nc.dram_tensor("v", (TC, F), BF16, kind="ExternalInput")
    vh = nc.dram_tensor("vh", (2 * VH, F), BF16, kind="ExternalInput")
    w1f = nc.dram_tensor("w1f", (F, F), FP8, kind="ExternalInput")
    w2f = nc.dram_tensor("w2f", (F, HC), FP8, kind="ExternalInput")
    w3f = nc.dram_tensor("w3f", (F, F), BF16, kind="ExternalInput")
    wof = nc.dram_tensor("wof", (F, F), BF16, kind="ExternalInput")
    out = nc.dram_tensor("out", (TC, F), BF16, kind="ExternalOutput")

    with tile.TileContext(nc) as tc:
        with tc.tile_pool(name="wpool", bufs=1) as wp, \
             tc.tile_pool(name="inpool", bufs=1) as inp, \
             tc.tile_pool(name="persist", bufs=1) as pers, \
             tc.tile_pool(name="work", bufs=2) as wk, \
             tc.tile_pool(name="band", bufs=4) as bp, \
             tc.tile_pool(name="psmain", bufs=2, space="PSUM") as psm, \
             tc.tile_pool(name="psband", bufs=4, space="PSUM") as psb, \
             tc.tile_pool(name="pstp", bufs=2, space="PSUM") as ptp, \
             tc.tile_pool(name="drampool", bufs=1, space="DRAM") as dp:

            # ---- weights to SBUF, [128, KF, n] layout (partition = contraction)
            w1_t = wp.tile([128, KF, F], FP8, tag="w1")
            nc.sync.dma_start(w1_t[:], w1f[:, :].rearrange("(ko p) n -> p ko n", p=128))
            w2_t = wp.tile([128, KF, HC], FP8, tag="w2")
            nc.sync.dma_start(w2_t[:], w2f[:, :].rearrange("(ko p) n -> p ko n", p=128))
            w3_t = wp.tile([128, KF, F], BF16, tag="w3")
            nc.sync.dma_start(w3_t[:], w3f[:, :].rearrange("(ko p) n -> p ko n", p=128))
            wo_t = wp.tile([128, KF, F], BF16, tag="wo")
            nc.sync.dma_start(wo_t[:], wof[:, :].rearrange("(ko p) n -> p ko n", p=128))

            # ---- v (t-major bf16): XBAR transpose to f-major
            # vT_t cols: [0, VH) front halo | [VH, VH+TC) main | back halo
            vT_t = inp.tile([128, KF, TV], BF16, tag="vT")
            vhT = inp.tile([128, KF, 2 * VH], BF16, tag="vhT")
            for fo in range(KF):
                eng = nc.scalar if fo % 2 else nc.sync
                eng.dma_start_transpose(vT_t[:, fo, VH:VH + TC],
                                        v[:, fo * 128:(fo + 1) * 128])
                eng.dma_start_transpose(vhT[:, fo, :],
                                        vh[:, fo * 128:(fo + 1) * 128])
            nc.vector.tensor_copy(out=vT_t[:, :, 0:VH], in_=vhT[:, :, 0:VH])
            nc.vector.tensor_copy(out=vT_t[:, :, VH + TC:TV],
                                  in_=vhT[:, :, VH:2 * VH])

            # ---- q (t-major fp8): PE-transpose to f-major
            ident = pers.tile([128, 128], FP8, tag="ident")
            masks.make_identity(nc, ident[:])
            qT_t = inp.tile([128, KF, TC], FP8, tag="qT")
            for tt in range(NT128):
                qstage = wk.tile([128, F], FP8, tag="qstage")
                nc.sync.dma_start(qstage[:], q[tt * 128:(tt + 1) * 128, :])
                for fo in range(KF):
                    # fp8 PE transpose requires output element step of 2
                    pst = ptp.tile([128, 256], FP8, tag="qtp")
                    pstv = pst[:].rearrange("p (a b) -> p a b", b=2)[:, :, 0]
                    nc.tensor.transpose(pstv,
                                        qstage[:, fo * 128:(fo + 1) * 128],
                                        ident[:])
                    nc.scalar.copy(qT_t[:, fo, tt * 128:(tt + 1) * 128],
                                   pstv)

            # ---- DRAM scratch
            # vproj rows j = w3-projection of v_in row j; chunk-vpad[r] = row
            # r + VOFF; v_in's zero halo rows project to exact zeros
            vproj = dp.tile([TV, F], BF16)
            # apad: 1 guard row + TC data rows + 1 guard row, row = [8 heads x 128]
            apad = dp.tile([TC + 2, H * W], BF16)

            # zero tile for apad guards
            z_t = pers.tile([128, H * W], BF16, tag="zt")
            nc.any.memzero(z_t[:])
            nc.sync.dma_start(apad[0:1, :], z_t[0:1, :])
            nc.sync.dma_start(apad[TC + 1:TC + 2, :], z_t[0:1, :])

            # ---- persistent SBUF activations
            qrT = pers.tile([128, KF, TC], FP8, tag="qrT")   # relu(q @ w1), f-major
            xT = pers.tile([128, KF, TC], BF16, tag="xT")    # band output, f-major

            # ================= Phase A: q-proj + relu (f-major out) ===========
            # PSUM = q @ (16 w1); Relu(psum/16) -> fp8
            for fo in range(KF):
                for tt in range(TC // 512):
                    ps = psm.tile([128, 512], F32, tag="mm")
                    for k in range(KF):
                        nc.tensor.matmul(
                            ps[:], w1_t[:, k, fo * 128:(fo + 1) * 128],
                            qT_t[:, k, tt * 512:(tt + 1) * 512],
                            start=(k == 0), stop=(k == KF - 1))
                    nc.scalar.activation(qrT[:, fo, tt * 512:(tt + 1) * 512], ps[:],
                                         mybir.ActivationFunctionType.Relu,
                                         scale=1.0 / WSCALE)

            # ================= Phase C: v-proj (t-major out) -> vproj =========
            for tb in range(NTV):
                ps = psm.tile([128, 512], F32, tag="mm")
                for k in range(KF):
                    nc.tensor.matmul(
                        ps[:], vT_t[:, k, tb * 128:(tb + 1) * 128],
                        w3_t[:, k, :],
                        start=(k == 0), stop=(k == KF - 1))
                v_sb = wk.tile([128, F], BF16, tag="vsb")
                nc.scalar.copy(v_sb[:], ps[:])
                nc.sync.dma_start(vproj[tb * 128:(tb + 1) * 128, :], v_sb[:])

            # ====== Phase B: s-proj (t-major) + softmax -> apad (padded) ======
            # PSUM = qr @ (16 w2); Exp(psum/16)
            for tb in range(NT128):
                ps = psm.tile([128, 512], F32, tag="mm")
                for k in range(KF):
                    nc.tensor.matmul(
                        ps[:, 0:HC], qrT[:, k, tb * 128:(tb + 1) * 128],
                        w2_t[:, k, :],
                        start=(k == 0), stop=(k == KF - 1))
                e_t = wk.tile([128, HC], F32, tag="et")
                nc.scalar.activation(e_t[:], ps[:, 0:HC],
                                     mybir.ActivationFunctionType.Exp,
                                     scale=1.0 / WSCALE)
                zs = wk.tile([128, H], F32, tag="zs")
                nc.vector.reduce_sum(zs[:], e_t[:].rearrange("p (h c) -> p h c", c=C),
                                     axis=mybir.AxisListType.X)
                rz = wk.tile([128, H], F32, tag="rz")
                nc.vector.reciprocal(rz[:], zs[:])
                ap_t = wk.tile([128, H * W], BF16, tag="apad")
                if tb < 2:
                    # zero the pad region once per pool slot (bufs=2); the pad
                    # columns are never overwritten afterwards
                    nc.any.memzero(ap_t[:])
                nc.vector.tensor_mul(
                    out=ap_t[:].rearrange("p (h w) -> p h w", w=W)[:, :, 0:C],
                    in0=e_t[:].rearrange("p (h c) -> p h c", c=C),
                    in1=rz[:, :, None].to_broadcast((128, H, C)))
                nc.sync.dma_start(apad[1 + tb * 128:1 + (tb + 1) * 128, :], ap_t[:])

            # ================= Phase D: banded attention matmuls ==============
            # x[t', h*64+d] = sum_s chunkvpad[t0+s, h*64+d] * B_h[s, t']
            # B_h loaded via transpose-DMA of sheared apad view.
            apad_h = apad.tensor  # underlying DRAM handle
            apad_off = apad.offset if isinstance(apad.offset, int) else 0
            for g in range(NB // 4):    # groups of 4 band blocks = 256 t'
                pss = [psb.tile([128, 512], F32, tag="px", name=f"px{g}_{pi}")
                       for pi in range(4)]
                for j in range(4):
                    bi = g * 4 + j
                    t0 = S * bi
                    vsp = wk.tile([128, F], BF16, tag="vsp")
                    nc.sync.dma_start(vsp[:], vproj[VOFF + t0:VOFF + t0 + 128, :])
                    for p in range(4):      # head pairs
                        for i in range(2):
                            h = 2 * p + i
                            b_t = bp.tile([W, S], BF16, tag="bt")
                            src = bass.AP(
                                tensor=apad_h,
                                offset=apad_off + (1 + t0) * (H * W) + h * W,
                                ap=[[H * W - 1, S], [1, W]])
                            eng = nc.scalar if h % 2 else nc.sync
                            eng.dma_start_transpose(b_t[:], src)
                            # lhsT = v head-pair [128, 128]; valid out rows are
                            # [i*64:(i+1)*64]; the other half is garbage and
                            # ignored at copyback.
                            nc.tensor.matmul(
                                pss[p][:, j * 128 + i * 64: j * 128 + (i + 1) * 64],
                                vsp[:, p * 128:(p + 1) * 128], b_t[:],
                                start=True, stop=True)
                # copy valid quadrants -> xT (f-major): fold p rows 0:63 = head
                # 2p (cols i=0), rows 64:127 = head 2p+1 (cols i=1)
                for p in range(4):
                    ps3 = pss[p][:].rearrange("d (j i k) -> d j i k", j=4, i=2)
                    dst = xT[:, p, g * 256:(g + 1) * 256] \
                        .rearrange("d (j k) -> d j k", j=4)
                    nc.vector.tensor_copy(out=dst[0:64], in_=ps3[0:64, :, 0, :])
                    nc.vector.tensor_copy(out=dst[64:128], in_=ps3[64:128, :, 1, :])

            # ================= Phase E: out-proj (t-major out) ================
            for tb in range(NT128):
                ps = psm.tile([128, 512], F32, tag="mm")
                for k in range(KF):
                    nc.tensor.matmul(
                        ps[:], xT[:, k, tb * 128:(tb + 1) * 128],
                        wo_t[:, k, :],
                        start=(k == 0), stop=(k == KF - 1))
                o_sb = wk.tile([128, F], BF16, tag="osb")
                nc.scalar.copy(o_sb[:], ps[:])
                nc.sync.dma_start(out[tb * 128:(tb + 1) * 128, :], o_sb[:])

    nc.compile()
    return nc


def _make_exec(nc, devices, donate_input=None):
    """Cached jitted executable + on-device zeros maker for one bass module.

    With donate_input=<name>, that input is donated and XLA aliases its
    buffer as the (shape/dtype-matching) output — no zero buffers needed."""
    import jax
    import jax.numpy as jnp
    from jax.sharding import Mesh, PartitionSpec, NamedSharding
    from jax.experimental.shard_map import shard_map
    from concourse.bass2jax import _bass_exec_p, partition_id_tensor

    partition_name = (nc.partition_id_tensor.name
                      if nc.partition_id_tensor else None)
    in_names, out_names, out_avals = [], [], []
    for alloc in nc.m.functions[0].allocations:
        if not isinstance(alloc, mybir.MemoryLocationSet):
            continue
        if alloc.kind not in ("ExternalInput", "ExternalOutput"):
            continue
        name = alloc.memorylocations[0].name
        if alloc.kind == "ExternalInput":
            if name != partition_name:
                in_names.append(name)
        else:
            out_avals.append(jax.core.ShapedArray(
                tuple(alloc.tensor_shape), mybir.dt.np(alloc.dtype)))
            out_names.append(name)
    n_params, n_outs = len(in_names), len(out_avals)
    in_names_all = list(in_names) + list(out_names)
    if partition_name is not None:
        in_names_all.append(partition_name)

    def _body(*args):
        operands = list(args)
        if partition_name is not None:
            operands.append(partition_id_tensor())
        return tuple(_bass_exec_p.bind(
            *operands,
            out_avals=tuple(out_avals),
            in_names=tuple(in_names_all),
            out_names=tuple(out_names),
            lowering_input_output_aliases=(),
            sim_require_finite=True,
            sim_require_nnan=True,
            nc=nc))

    n = len(devices)
    mesh = Mesh(np.asarray(devices), ("core",))
    shard = NamedSharding(mesh, PartitionSpec("core"))
    if donate_input is None:
        n_args = n_params + n_outs
        donate = tuple(range(n_params, n_args))
        mkzeros = jax.jit(
            lambda: tuple(jnp.zeros((n * a.shape[0], *a.shape[1:]), a.dtype)
                          for a in out_avals),
            out_shardings=(shard,) * n_outs)
        body = _body
    else:
        # outputs alias the donated input's buffer; no zero operands
        n_args = n_params
        donate = (in_names.index(donate_input),)
        mkzeros = None
        in_names_all[:] = list(in_names)
        if partition_name is not None:
            in_names_all.append(partition_name)
        body = _body
    in_specs = (PartitionSpec("core"),) * n_args
    out_specs = (PartitionSpec("core"),) * n_outs
    sharded = jax.jit(
        shard_map(body, mesh=mesh, in_specs=in_specs, out_specs=out_specs,
                  check_rep=False),
        donate_argnums=donate, keep_unused=True)
    return {"sharded": sharded, "mkzeros": mkzeros, "in_names": in_names,
            "out_names": out_names, "shard": shard, "n": n}


def _get_state():
    if "state" in _CACHE:
        return _CACHE["state"]
    import jax
    from concourse.bass2jax import install_neuronx_cc_hook
    install_neuronx_cc_hook()
    devices = jax.devices()[:B]
    wexec = _make_exec(_build_w(), devices)
    kexec = _make_exec(_build_k(), devices, donate_input="v")
    state = {"w": wexec, "k": kexec}
    _CACHE["state"] = state
    return state


def _to_bf16_bits(x32):
    """fp32 -> bf16 via round-half-up on the upper 16 bits (RNE-grade error,
    much faster than ml_dtypes astype). Returns uint16 bit pattern."""
    v = np.ascontiguousarray(x32).view(np.uint32)
    return np.right_shift(v + np.uint32(0x8000), 16).astype(np.uint16)


def kernel(query, key, value, w1, w2, w3, w_out, _trace=False):
    import jax
    st = _get_state()
    e4 = ml_dtypes.float8_e4m3
    bf = ml_dtypes.bfloat16
    wx, kx = st["w"], st["k"]

    query = np.asarray(query)
    value = np.asarray(value)

    # ---- weights launch first: tiny upload, runs while v/q upload
    wzeros = wx["mkzeros"]()
    warrs = {"w1s": (np.asarray(w1) * WSCALE).astype(e4),
             "w2s": (np.asarray(w2) * WSCALE).astype(e4),
             "w3s": _to_bf16_bits(np.asarray(w3)).view(bf),
             "wos": _to_bf16_bits(np.asarray(w_out)).view(bf)}
    wouts = wx["sharded"](*[warrs[n] for n in wx["in_names"]], *wzeros)
    wfull = dict(zip(wx["out_names"], wouts))

    # ---- per-chunk compute launches, pipelined
    pending = []
    for ci in range(NCHUNK):
        c0 = ci * TC
        # v main part (donated: its buffer is reused as the output)
        vb = _to_bf16_bits(value[:, c0:c0 + TC]).view(bf).reshape(B * TC, F)
        v_dev = jax.device_put(vb, kx["shard"])
        # v halo rows (zero-padded at sequence edges)
        hbuf = np.zeros((B, 2 * VH, F), np.uint16)
        if c0 > 0:
            hbuf[:, 0:VH] = _to_bf16_bits(value[:, c0 - VH:c0])
        if c0 + TC < T:
            hbuf[:, VH:2 * VH] = _to_bf16_bits(value[:, c0 + TC:c0 + TC + VH])
        vh_dev = jax.device_put(hbuf.view(bf).reshape(B * 2 * VH, F),
                                kx["shard"])
        # q chunk, t-major fp8 (transposed on device)
        q8 = query[:, c0:c0 + TC, :].astype(e4).reshape(B * TC, F)
        q_dev = jax.device_put(q8, kx["shard"])
        arrays = {"q": q_dev, "v": v_dev, "vh": vh_dev, "w1f": wfull["w1f"],
                  "w2f": wfull["w2f"], "w3f": wfull["w3f"],
                  "wof": wfull["wof"]}
        ins = [arrays[n] for n in kx["in_names"]]
        pending.append(kx["sharded"](*ins))

    # ---- collect: widen bf16 -> fp32 exactly (zero-extension)
    buf = np.zeros((B, T, F, 2), np.uint16)
    for ci, outs in enumerate(pending):
        c0 = ci * TC
        o16 = np.asarray(outs[0]).view(np.uint16).reshape(B, TC, F)
        buf[:, c0:c0 + TC, :, 1] = o16
    return buf.view(np.float32)[..., 0]


# revision 24
# speedup vs baseline: 1.1464x; 1.0353x over previous
"""Trainium2 Bass kernel for LocalDenseSynthesizerAttention.

Data-parallel over batch B=8 -> 8 cores, one batch each. Wire-traffic and
dispatch optimized for the axon tunnel (~90MB/s each way, full duplex):
  - jitted executables built once and cached (no per-call retrace)
  - q shipped t-major fp8 (e4m3) and transposed on-device (PE transpose);
    v shipped t-major bf16 and transposed on-device via XBAR DMA;
    output returned bf16 and widened exactly on host
  - w1/w2 shipped fp8 scaled x16 (rescaled on device via activation scale),
    w3/w_out bf16; shipped as 8-way shards once per call to a tiny
    weights launch that AllGathers them on device; the full per-core
    weights stay device-resident and feed the compute launches
  - compute is split into sequence chunks (the attention window is local,
    halo = 22), one 8-core launch per chunk: chunk i+1's upload overlaps
    chunk i's exec + download
  - donated output buffers created on-device (no zeros upload)

The local window C=45 weighted sum is computed as banded matmuls: the banded
matrix B[s,t'] = attn[t0+t',h,s-t'] is an affine strided view of a zero-padded
attn tensor in DRAM, loaded matmul-ready via XBAR transpose-DMA.

Self-contained: hardcodes shapes from the problem spec.
"""
import sys
sys.path.insert(0, '/opt/trn_rl_repo')
import numpy as np
import ml_dtypes

import concourse.bass as bass
import concourse.mybir as mybir
import concourse.tile as tile
from concourse import bacc
from concourse import masks

T, F = 2048, 512
H, C, DK = 8, 45, 64
HC = H * C          # 360
W = 128             # padded attn width per head (covers s-t' in [-63,127])
S = 64              # t' band-block size
PADV = 22           # (C-1)//2
KF = F // 128       # 4 contraction chunks
B = 8               # total batches / cores
FSH = F // B        # 64 weight-shard rows per core

NCHUNK = 2
TC = T // NCHUNK    # sequence-chunk length
VH = 64             # v halo rows each side (>= PADV, keeps tiles 128-aligned)
TV = TC + 2 * VH    # logical v rows per chunk (main + halo)
VOFF = VH - PADV    # chunk-vpad[r] = v_logical[r + VOFF]

BF16 = mybir.dt.bfloat16
FP8 = mybir.dt.float8e4
F32 = mybir.dt.float32
WSCALE = 16.0       # fp8 weight pre-scale for w1/w2

_CACHE = {}


def _build_w():
    """Tiny weights launch: AllGather 8-way weight shards into full
    per-core weights (device-resident outputs)."""
    nc = bacc.Bacc("TRN2", target_bir_lowering=False, debug=False,
                   num_devices=B)
    w1s = nc.dram_tensor("w1s", (FSH, F), FP8, kind="ExternalInput")
    w2s = nc.dram_tensor("w2s", (FSH, HC), FP8, kind="ExternalInput")
    w3s = nc.dram_tensor("w3s", (FSH, F), BF16, kind="ExternalInput")
    wos = nc.dram_tensor("wos", (FSH, F), BF16, kind="ExternalInput")
    w1f = nc.dram_tensor("w1f", (F, F), FP8, kind="ExternalOutput")
    w2f = nc.dram_tensor("w2f", (F, HC), FP8, kind="ExternalOutput")
    w3f = nc.dram_tensor("w3f", (F, F), BF16, kind="ExternalOutput")
    wof = nc.dram_tensor("wof", (F, F), BF16, kind="ExternalOutput")
    groups = [list(range(B))]
    with tile.TileContext(nc) as tc:
        with tc.tile_pool(name="dram", bufs=1, space="DRAM") as dp:
            # collectives cannot read IO tensors: stage shards first
            stages = (dp.tile([FSH, F], FP8, name="st1"),
                      dp.tile([FSH, HC], FP8, name="st2"),
                      dp.tile([FSH, F], BF16, name="st3"),
                      dp.tile([FSH, F], BF16, name="st4"))
            fulls = (dp.tile([F, F], FP8, name="fu1"),
                     dp.tile([F, HC], FP8, name="fu2"),
                     dp.tile([F, F], BF16, name="fu3"),
                     dp.tile([F, F], BF16, name="fu4"))
            for shard, stage, full, out in zip(
                    (w1s, w2s, w3s, wos), stages, fulls,
                    (w1f, w2f, w3f, wof)):
                nc.sync.dma_start(stage[:, :], shard[:, :])
                nc.gpsimd.collective_compute(
                    "AllGather", mybir.AluOpType.bypass, groups,
                    [stage[:, :]], [full[:, :]])
                nc.sync.dma_start(out[:, :], full[:, :])
    nc.compile()
    return nc


def _build_k():
    """Compute launch for one sequence chunk of TC rows."""
    NT128 = TC // 128           # t-tiles in the chunk
    NTV = TV // 128             # t-tiles of the v input (incl halo)
    NB = TC // S                # band blocks
    nc = bacc.Bacc("TRN2", target_bir_lowering=False, debug=False,
                   num_devices=B)
    q = nc.dram_tensor("q", (TC, F), FP8, kind="ExternalInput")
    # v is split so the main part exactly matches the output shape/dtype and
    # can be donated/aliased as the output buffer (saves a zeros launch):
    # vh rows [0, VH) = rows just before the chunk, [VH, 2VH) = just after
    v = nc.dram_tensor("v", (TC, F), BF16, kind="ExternalInput")
    vh = nc.dram_tensor("vh", (2 * VH, F), BF16, kind="ExternalInput")
    w1f = nc.dram_tensor("w1f", (F, F), FP8, kind="ExternalInput")
    w2f = nc.dram_tensor("w2f", (F, HC), FP8, kind="ExternalInput")
    w3f = nc.dram_tensor("w3f", (F, F), BF16, kind="ExternalInput")
    wof = nc.dram_tensor("wof", (F, F), BF16, kind="ExternalInput")
    out = nc.dram_tensor("out", (TC, F), BF16, kind="ExternalOutput")

    with tile.TileContext(nc) as tc:
        with tc.tile_pool(name="wpool", bufs=1) as wp, \
             tc.tile_pool(name="inpool", bufs=1) as inp, \
             tc.tile_pool(name="persist", bufs=1) as pers, \
             tc.tile_pool(name="work", bufs=2) as wk, \
             tc.tile_pool(name="band", bufs=4) as bp, \
             tc.tile_pool(name="psmain", bufs=2, space="PSUM") as psm, \
             tc.tile_pool(name="psband", bufs=4, space="PSUM") as psb, \
             tc.tile_pool(name="pstp", bufs=2, space="PSUM") as ptp, \
             tc.tile_pool(name="drampool", bufs=1, space="DRAM") as dp:

            # ---- weights to SBUF, [128, KF, n] layout (partition = contraction)
            w1_t = wp.tile([128, KF, F], FP8, tag="w1")
            nc.sync.dma_start(w1_t[:], w1f[:, :].rearrange("(ko p) n -> p ko n", p=128))
            w2_t = wp.tile([128, KF, HC], FP8, tag="w2")
            nc.sync.dma_start(w2_t[:], w2f[:, :].rearrange("(ko p) n -> p ko n", p=128))
            w3_t = wp.tile([128, KF, F], BF16, tag="w3")
            nc.sync.dma_start(w3_t[:], w3f[:, :].rearrange("(ko p) n -> p ko n", p=128))
            wo_t = wp.tile([128, KF, F], BF16, tag="wo")
            nc.sync.dma_start(wo_t[:], wof[:, :].rearrange("(ko p) n -> p ko n", p=128))

            # ---- v (t-major bf16): XBAR transpose to f-major
            # vT_t cols: [0, VH) front halo | [VH, VH+TC) main | back halo
            vT_t = inp.tile([128, KF, TV], BF16, tag="vT")
            vhT = inp.tile([128, KF, 2 * VH], BF16, tag="vhT")
            for fo in range(KF):
                eng = nc.scalar if fo % 2 else nc.sync
                eng.dma_start_transpose(vT_t[:, fo, VH:VH + TC],
                                        v[:, fo * 128:(fo + 1) * 128])
                eng.dma_start_transpose(vhT[:, fo, :],
                                        vh[:, fo * 128:(fo + 1) * 128])
            nc.vector.tensor_copy(out=vT_t[:, :, 0:VH], in_=vhT[:, :, 0:VH])
            nc.vector.tensor_copy(out=vT_t[:, :, VH + TC:TV],
                                  in_=vhT[:, :, VH:2 * VH])

            # ---- q (t-major fp8): PE-transpose to f-major
            ident = pers.tile([128, 128], FP8, tag="ident")
            masks.make_identity(nc, ident[:])
            qT_t = inp.tile([128, KF, TC], FP8, tag="qT")
            for tt in range(NT128):
                qstage = wk.tile([128, F], FP8, tag="qstage")
                nc.sync.dma_start(qstage[:], q[tt * 128:(tt + 1) * 128, :])
                for fo in range(KF):
                    # fp8 PE transpose requires output element step of 2
                    pst = ptp.tile([128, 256], FP8, tag="qtp")
                    pstv = pst[:].rearrange("p (a b) -> p a b", b=2)[:, :, 0]
                    nc.tensor.transpose(pstv,
                                        qstage[:, fo * 128:(fo + 1) * 128],
                                        ident[:])
                    nc.scalar.copy(qT_t[:, fo, tt * 128:(tt + 1) * 128],
                                   pstv)

            # ---- DRAM scratch
            # vproj rows j = w3-projection of v_in row j; chunk-vpad[r] = row
            # r + VOFF; v_in's zero halo rows project to exact zeros
            vproj = dp.tile([TV, F], BF16)
            # apad: 1 guard row + TC data rows + 1 guard row, row = [8 heads x 128]
            apad = dp.tile([TC + 2, H * W], BF16)

            # zero tile for apad guards
            z_t = pers.tile([128, H * W], BF16, tag="zt")
            nc.any.memzero(z_t[:])
            nc.sync.dma_start(apad[0:1, :], z_t[0:1, :])
            nc.sync.dma_start(apad[TC + 1:TC + 2, :], z_t[0:1, :])

            # ---- persistent SBUF activations
            qrT = pers.tile([128, KF, TC], FP8, tag="qrT")   # relu(q @ w1), f-major
            xT = pers.tile([128, KF, TC], BF16, tag="xT")    # band output, f-major

            # ================= Phase A: q-proj + relu (f-major out) ===========
            # PSUM = q @ (16 w1); Relu(psum/16) -> fp8
            for fo in range(KF):
                for tt in range(TC // 512):
                    ps = psm.tile([128, 512], F32, tag="mm")
                    for k in range(KF):
                        nc.tensor.matmul(
                            ps[:], w1_t[:, k, fo * 128:(fo + 1) * 128],
                            qT_t[:, k, tt * 512:(tt + 1) * 512],
                            start=(k == 0), stop=(k == KF - 1))
                    nc.scalar.activation(qrT[:, fo, tt * 512:(tt + 1) * 512], ps[:],
                                         mybir.ActivationFunctionType.Relu,
                                         scale=1.0 / WSCALE)

            # ================= Phase C: v-proj (t-major out) -> vproj =========
            for tb in range(NTV):
                ps = psm.tile([128, 512], F32, tag="mm")
                for k in range(KF):
                    nc.tensor.matmul(
                        ps[:], vT_t[:, k, tb * 128:(tb + 1) * 128],
                        w3_t[:, k, :],
                        start=(k == 0), stop=(k == KF - 1))
                v_sb = wk.tile([128, F], BF16, tag="vsb")
                nc.scalar.copy(v_sb[:], ps[:])
                nc.sync.dma_start(vproj[tb * 128:(tb + 1) * 128, :], v_sb[:])

            # ====== Phase B: s-proj (t-major) + softmax -> apad (padded) ======
            # PSUM = qr @ (16 w2); Exp(psum/16)
            for tb in range(NT128):
                ps = psm.tile([128, 512], F32, tag="mm")
                for k in range(KF):
                    nc.tensor.matmul(
                        ps[:, 0:HC], qrT[:, k, tb * 128:(tb + 1) * 128],
                        w2_t[:, k, :],
                        start=(k == 0), stop=(k == KF - 1))
                e_t = wk.tile([128, HC], F32, tag="et")
                nc.scalar.activation(e_t[:], ps[:, 0:HC],
                                     mybir.ActivationFunctionType.Exp,
                                     scale=1.0 / WSCALE)
                zs = wk.tile([128, H], F32, tag="zs")
                nc.vector.reduce_sum(zs[:], e_t[:].rearrange("p (h c) -> p h c", c=C),
                                     axis=mybir.AxisListType.X)
                rz = wk.tile([128, H], F32, tag="rz")
                nc.vector.reciprocal(rz[:], zs[:])
                ap_t = wk.tile([128, H * W], BF16, tag="apad")
                if tb < 2:
                    # zero the pad region once per pool slot (bufs=2); the pad
                    # columns are never overwritten afterwards
                    nc.any.memzero(ap_t[:])
                nc.vector.tensor_mul(
                    out=ap_t[:].rearrange("p (h w) -> p h w", w=W)[:, :, 0:C],
                    in0=e_t[:].rearrange("p (h c) -> p h c", c=C),
                    in1=rz[:, :, None].to_broadcast((128, H, C)))
                nc.sync.dma_start(apad[1 + tb * 128:1 + (tb + 1) * 128, :], ap_t[:])

            # ================= Phase D: banded attention matmuls ==============
            # x[t', h*64+d] = sum_s chunkvpad[t0+s, h*64+d] * B_h[s, t']
            # B_h loaded via transpose-DMA of sheared apad view.
            apad_h = apad.tensor  # underlying DRAM handle
            apad_off = apad.offset if isinstance(apad.offset, int) else 0
            for g in range(NB // 4):    # groups of 4 band blocks = 256 t'
                pss = [psb.tile([128, 512], F32, tag="px", name=f"px{g}_{pi}")
                       for pi in range(4)]
                for j in range(4):
                    bi = g * 4 + j
                    t0 = S * bi
                    vsp = wk.tile([128, F], BF16, tag="vsp")
                    nc.sync.dma_start(vsp[:], vproj[VOFF + t0:VOFF + t0 + 128, :])
                    for p in range(4):      # head pairs
                        for i in range(2):
                            h = 2 * p + i
                            b_t = bp.tile([W, S], BF16, tag="bt")
                            src = bass.AP(
                                tensor=apad_h,
                                offset=apad_off + (1 + t0) * (H * W) + h * W,
                                ap=[[H * W - 1, S], [1, W]])
                            eng = nc.scalar if h % 2 else nc.sync
                            eng.dma_start_transpose(b_t[:], src)
                            # lhsT = v head-pair [128, 128]; valid out rows are
                            # [i*64:(i+1)*64]; the other half is garbage and
                            # ignored at copyback.
                            nc.tensor.matmul(
                                pss[p][:, j * 128 + i * 64: j * 128 + (i + 1) * 64],
                                vsp[:, p * 128:(p + 1) * 128], b_t[:],
                                start=True, stop=True)
                # copy valid quadrants -> xT (f-major): fold p rows 0:63 = head
                # 2p (cols i=0), rows 64:127 = head 2p+1 (cols i=1)
                for p in range(4):
                    ps3 = pss[p][:].rearrange("d (j i k) -> d j i k", j=4, i=2)
                    dst = xT[:, p, g * 256:(g + 1) * 256] \
                        .rearrange("d (j k) -> d j k", j=4)
                    nc.vector.tensor_copy(out=dst[0:64], in_=ps3[0:64, :, 0, :])
                    nc.vector.tensor_copy(out=dst[64:128], in_=ps3[64:128, :, 1, :])

            # ================= Phase E: out-proj (t-major out) ================
            for tb in range(NT128):
                ps = psm.tile([128, 512], F32, tag="mm")
                for k in range(KF):
                    nc.tensor.matmul(
                        ps[:], xT[:, k, tb * 128:(tb + 1) * 128],
                        wo_t[:, k, :],
                        start=(k == 0), stop=(k == KF - 1))
                o_sb = wk.tile([128, F], BF16, tag="osb")
                nc.scalar.copy(o_sb[:], ps[:])
                nc.sync.dma_start(out[tb * 128:(tb + 1) * 128, :], o_sb[:])

    nc.compile()
    return nc


def _make_exec(nc, devices, donate_input=None):
    """Cached jitted executable + on-device zeros maker for one bass module.

    With donate_input=<name>, that input is donated and XLA aliases its
    buffer as the (shape/dtype-matching) output — no zero buffers needed."""
    import jax
    import jax.numpy as jnp
    from jax.sharding import Mesh, PartitionSpec, NamedSharding
    from jax.experimental.shard_map import shard_map
    from concourse.bass2jax import _bass_exec_p, partition_id_tensor

    partition_name = (nc.partition_id_tensor.name
                      if nc.partition_id_tensor else None)
    in_names, out_names, out_avals = [], [], []
    for alloc in nc.m.functions[0].allocations:
        if not isinstance(alloc, mybir.MemoryLocationSet):
            continue
        if alloc.kind not in ("ExternalInput", "ExternalOutput"):
            continue
        name = alloc.memorylocations[0].name
        if alloc.kind == "ExternalInput":
            if name != partition_name:
                in_names.append(name)
        else:
            out_avals.append(jax.core.ShapedArray(
                tuple(alloc.tensor_shape), mybir.dt.np(alloc.dtype)))
            out_names.append(name)
    n_params, n_outs = len(in_names), len(out_avals)
    in_names_all = list(in_names) + list(out_names)
    if partition_name is not None:
        in_names_all.append(partition_name)

    def _body(*args):
        operands = list(args)
        if partition_name is not None:
            operands.append(partition_id_tensor())
        return tuple(_bass_exec_p.bind(
            *operands,
            out_avals=tuple(out_avals),
            in_names=tuple(in_names_all),
            out_names=tuple(out_names),
            lowering_input_output_aliases=(),
            sim_require_finite=True,
            sim_require_nnan=True,
            nc=nc))

    n = len(devices)
    mesh = Mesh(np.asarray(devices), ("core",))
    shard = NamedSharding(mesh, PartitionSpec("core"))
    if donate_input is None:
        n_args = n_params + n_outs
        donate = tuple(range(n_params, n_args))
        mkzeros = jax.jit(
            lambda: tuple(jnp.zeros((n * a.shape[0], *a.shape[1:]), a.dtype)
                          for a in out_avals),
            out_shardings=(shard,) * n_outs)
        body = _body
    else:
        # outputs alias the donated input's buffer; no zero operands
        n_args = n_params
        donate = (in_names.index(donate_input),)
        mkzeros = None
        in_names_all[:] = list(in_names)
        if partition_name is not None:
            in_names_all.append(partition_name)
        body = _body
    in_specs = (PartitionSpec("core"),) * n_args
    out_specs = (PartitionSpec("core"),) * n_outs
    sharded = jax.jit(
        shard_map(body, mesh=mesh, in_specs=in_specs, out_specs=out_specs,
                  check_rep=False),
        donate_argnums=donate, keep_unused=True)
    return {"sharded": sharded, "mkzeros": mkzeros, "in_names": in_names,
            "out_names": out_names, "shard": shard, "n": n}


def _get_state():
    if "state" in _CACHE:
        return _CACHE["state"]
    import jax
    from concourse.bass2jax import install_neuronx_cc_hook
    install_neuronx_cc_hook()
    devices = jax.devices()[:B]
    wexec = _make_exec(_build_w(), devices)
    kexec = _make_exec(_build_k(), devices, donate_input="v")
    state = {"w": wexec, "k": kexec}
    _CACHE["state"] = state
    return state


def _to_bf16_bits(x32):
    """fp32 -> bf16 via round-half-up on the upper 16 bits (RNE-grade error,
    much faster than ml_dtypes astype). Returns uint16 bit pattern."""
    v = np.ascontiguousarray(x32).view(np.uint32)
    return np.right_shift(v + np.uint32(0x8000), 16).astype(np.uint16)


def _conv_chunk(query, value, ci):
    """Convert one chunk's inputs (numpy, GIL-releasing ops; run in pool)."""
    e4 = ml_dtypes.float8_e4m3
    bf = ml_dtypes.bfloat16
    c0 = ci * TC
    vb = _to_bf16_bits(value[:, c0:c0 + TC]).view(bf).reshape(B * TC, F)
    hbuf = np.zeros((B, 2 * VH, F), np.uint16)
    if c0 > 0:
        hbuf[:, 0:VH] = _to_bf16_bits(value[:, c0 - VH:c0])
    if c0 + TC < T:
        hbuf[:, VH:2 * VH] = _to_bf16_bits(value[:, c0 + TC:c0 + TC + VH])
    hb = hbuf.view(bf).reshape(B * 2 * VH, F)
    q8 = query[:, c0:c0 + TC, :].astype(e4).reshape(B * TC, F)
    return vb, hb, q8


def kernel(query, key, value, w1, w2, w3, w_out, _trace=False):
    import jax
    from concurrent.futures import ThreadPoolExecutor
    st = _get_state()
    e4 = ml_dtypes.float8_e4m3
    bf = ml_dtypes.bfloat16
    wx, kx = st["w"], st["k"]

    query = np.asarray(query)
    value = np.asarray(value)

    # convert all chunks in parallel with the uploads below
    pool = _CACHE.setdefault("pool", ThreadPoolExecutor(max_workers=NCHUNK))
    futs = [pool.submit(_conv_chunk, query, value, ci)
            for ci in range(NCHUNK)]

    # ---- weights launch first: tiny upload, runs while v/q upload
    wzeros = wx["mkzeros"]()
    warrs = {"w1s": (np.asarray(w1) * WSCALE).astype(e4),
             "w2s": (np.asarray(w2) * WSCALE).astype(e4),
             "w3s": _to_bf16_bits(np.asarray(w3)).view(bf),
             "wos": _to_bf16_bits(np.asarray(w_out)).view(bf)}
    wouts = wx["sharded"](*[warrs[n] for n in wx["in_names"]], *wzeros)
    wfull = dict(zip(wx["out_names"], wouts))

    # ---- per-chunk compute launches, pipelined
    pending = []
    for ci in range(NCHUNK):
        vb, hb, q8 = futs[ci].result()
        v_dev = jax.device_put(vb, kx["shard"])     # donated -> output buffer
        vh_dev = jax.device_put(hb, kx["shard"])
        q_dev = jax.device_put(q8, kx["shard"])
        arrays = {"q": q_dev, "v": v_dev, "vh": vh_dev, "w1f": wfull["w1f"],
                  "w2f": wfull["w2f"], "w3f": wfull["w3f"],
                  "wof": wfull["wof"]}
        ins = [arrays[n] for n in kx["in_names"]]
        pending.append(kx["sharded"](*ins))

    # ---- collect: widen bf16 -> fp32 exactly (zero-extension)
    buf = np.zeros((B, T, F, 2), np.uint16)
    for ci, outs in enumerate(pending):
        c0 = ci * TC
        o16 = np.asarray(outs[0]).view(np.uint16).reshape(B, TC, F)
        buf[:, c0:c0 + TC, :, 1] = o16
    return buf.view(np.float32)[..., 0]


# revision 26
# speedup vs baseline: 1.2397x; 1.0814x over previous
"""Trainium2 Bass kernel for LocalDenseSynthesizerAttention.

Data-parallel over batch B=8 -> 8 cores, one batch each. Wire-traffic and
dispatch optimized for the axon tunnel (~90MB/s each way, full duplex):
  - jitted executables built once and cached (no per-call retrace)
  - q shipped t-major fp8 (e4m3) and transposed on-device (PE transpose);
    v shipped t-major bf16 and transposed on-device via XBAR DMA;
    output returned bf16 and widened exactly on host
  - w1/w2 shipped fp8 scaled x16 (rescaled on device via activation scale),
    w3/w_out bf16; shipped as 8-way shards once per call to a tiny
    weights launch that AllGathers them on device; the full per-core
    weights stay device-resident and feed the compute launches
  - compute is split into sequence chunks (the attention window is local,
    halo = 22), one 8-core launch per chunk: chunk i+1's upload overlaps
    chunk i's exec + download
  - donated output buffers created on-device (no zeros upload)

The local window C=45 weighted sum is computed as banded matmuls: the banded
matrix B[s,t'] = attn[t0+t',h,s-t'] is an affine strided view of a zero-padded
attn tensor in DRAM, loaded matmul-ready via XBAR transpose-DMA.

Self-contained: hardcodes shapes from the problem spec.
"""
import sys
sys.path.insert(0, '/opt/trn_rl_repo')
import numpy as np
import ml_dtypes

import concourse.bass as bass
import concourse.mybir as mybir
import concourse.tile as tile
from concourse import bacc
from concourse import masks

T, F = 2048, 512
H, C, DK = 8, 45, 64
HC = H * C          # 360
W = 128             # padded attn width per head (covers s-t' in [-63,127])
S = 64              # t' band-block size
PADV = 22           # (C-1)//2
KF = F // 128       # 4 contraction chunks
B = 8               # total batches / cores
FSH = F // B        # 64 weight-shard rows per core

NCHUNK = 2
TC = T // NCHUNK    # sequence-chunk length
VH = 64             # v halo rows each side (>= PADV, keeps tiles 128-aligned)
TV = TC + 2 * VH    # logical v rows per chunk (main + halo)
VOFF = VH - PADV    # chunk-vpad[r] = v_logical[r + VOFF]

BF16 = mybir.dt.bfloat16
FP8 = mybir.dt.float8e4
F32 = mybir.dt.float32
WSCALE = 16.0       # fp8 weight pre-scale for w1/w2

_CACHE = {}


def _build_w():
    """Tiny weights launch: AllGather 8-way weight shards into full
    per-core weights (device-resident outputs)."""
    nc = bacc.Bacc("TRN2", target_bir_lowering=False, debug=False,
                   num_devices=B)
    w1s = nc.dram_tensor("w1s", (FSH, F), FP8, kind="ExternalInput")
    w2s = nc.dram_tensor("w2s", (FSH, HC), FP8, kind="ExternalInput")
    w3s = nc.dram_tensor("w3s", (FSH, F), BF16, kind="ExternalInput")
    wos = nc.dram_tensor("wos", (FSH, F), BF16, kind="ExternalInput")
    w1f = nc.dram_tensor("w1f", (F, F), FP8, kind="ExternalOutput")
    w2f = nc.dram_tensor("w2f", (F, HC), FP8, kind="ExternalOutput")
    w3f = nc.dram_tensor("w3f", (F, F), BF16, kind="ExternalOutput")
    wof = nc.dram_tensor("wof", (F, F), BF16, kind="ExternalOutput")
    groups = [list(range(B))]
    with tile.TileContext(nc) as tc:
        with tc.tile_pool(name="dram", bufs=1, space="DRAM") as dp:
            # collectives cannot read IO tensors: stage shards first
            stages = (dp.tile([FSH, F], FP8, name="st1"),
                      dp.tile([FSH, HC], FP8, name="st2"),
                      dp.tile([FSH, F], BF16, name="st3"),
                      dp.tile([FSH, F], BF16, name="st4"))
            fulls = (dp.tile([F, F], FP8, name="fu1"),
                     dp.tile([F, HC], FP8, name="fu2"),
                     dp.tile([F, F], BF16, name="fu3"),
                     dp.tile([F, F], BF16, name="fu4"))
            for shard, stage, full, out in zip(
                    (w1s, w2s, w3s, wos), stages, fulls,
                    (w1f, w2f, w3f, wof)):
                nc.sync.dma_start(stage[:, :], shard[:, :])
                nc.gpsimd.collective_compute(
                    "AllGather", mybir.AluOpType.bypass, groups,
                    [stage[:, :]], [full[:, :]])
                nc.sync.dma_start(out[:, :], full[:, :])
    nc.compile()
    return nc


def _build_k():
    """Compute launch for one sequence chunk of TC rows."""
    NT128 = TC // 128           # t-tiles in the chunk
    NTV = TV // 128             # t-tiles of the v input (incl halo)
    NB = TC // S                # band blocks
    nc = bacc.Bacc("TRN2", target_bir_lowering=False, debug=False,
                   num_devices=B)
    q = nc.dram_tensor("q", (TC, F), FP8, kind="ExternalInput")
    # v is split so the main part exactly matches the output shape/dtype and
    # can be donated/aliased as the output buffer (saves a zeros launch):
    # vh rows [0, VH) = rows just before the chunk, [VH, 2VH) = just after
    v = nc.dram_tensor("v", (TC, F), BF16, kind="ExternalInput")
    vh = nc.dram_tensor("vh", (2 * VH, F), BF16, kind="ExternalInput")
    w1f = nc.dram_tensor("w1f", (F, F), FP8, kind="ExternalInput")
    w2f = nc.dram_tensor("w2f", (F, HC), FP8, kind="ExternalInput")
    w3f = nc.dram_tensor("w3f", (F, F), BF16, kind="ExternalInput")
    wof = nc.dram_tensor("wof", (F, F), BF16, kind="ExternalInput")
    out = nc.dram_tensor("out", (TC, F), BF16, kind="ExternalOutput")

    with tile.TileContext(nc) as tc:
        with tc.tile_pool(name="wpool", bufs=1) as wp, \
             tc.tile_pool(name="inpool", bufs=1) as inp, \
             tc.tile_pool(name="persist", bufs=1) as pers, \
             tc.tile_pool(name="work", bufs=2) as wk, \
             tc.tile_pool(name="band", bufs=4) as bp, \
             tc.tile_pool(name="psmain", bufs=2, space="PSUM") as psm, \
             tc.tile_pool(name="psband", bufs=4, space="PSUM") as psb, \
             tc.tile_pool(name="pstp", bufs=2, space="PSUM") as ptp, \
             tc.tile_pool(name="drampool", bufs=1, space="DRAM") as dp:

            # ---- weights to SBUF, [128, KF, n] layout (partition = contraction)
            w1_t = wp.tile([128, KF, F], FP8, tag="w1")
            nc.sync.dma_start(w1_t[:], w1f[:, :].rearrange("(ko p) n -> p ko n", p=128))
            w2_t = wp.tile([128, KF, HC], FP8, tag="w2")
            nc.sync.dma_start(w2_t[:], w2f[:, :].rearrange("(ko p) n -> p ko n", p=128))
            w3_t = wp.tile([128, KF, F], BF16, tag="w3")
            nc.sync.dma_start(w3_t[:], w3f[:, :].rearrange("(ko p) n -> p ko n", p=128))
            wo_t = wp.tile([128, KF, F], BF16, tag="wo")
            nc.sync.dma_start(wo_t[:], wof[:, :].rearrange("(ko p) n -> p ko n", p=128))

            # ---- v (t-major bf16): XBAR transpose to f-major
            # vT_t cols: [0, VH) front halo | [VH, VH+TC) main | back halo
            vT_t = inp.tile([128, KF, TV], BF16, tag="vT")
            vhT = inp.tile([128, KF, 2 * VH], BF16, tag="vhT")
            for fo in range(KF):
                eng = nc.scalar if fo % 2 else nc.sync
                eng.dma_start_transpose(vT_t[:, fo, VH:VH + TC],
                                        v[:, fo * 128:(fo + 1) * 128])
                eng.dma_start_transpose(vhT[:, fo, :],
                                        vh[:, fo * 128:(fo + 1) * 128])
            nc.vector.tensor_copy(out=vT_t[:, :, 0:VH], in_=vhT[:, :, 0:VH])
            nc.vector.tensor_copy(out=vT_t[:, :, VH + TC:TV],
                                  in_=vhT[:, :, VH:2 * VH])

            # ---- q (t-major fp8): PE-transpose to f-major
            ident = pers.tile([128, 128], FP8, tag="ident")
            masks.make_identity(nc, ident[:])
            qT_t = inp.tile([128, KF, TC], FP8, tag="qT")
            for tt in range(NT128):
                qstage = wk.tile([128, F], FP8, tag="qstage")
                nc.sync.dma_start(qstage[:], q[tt * 128:(tt + 1) * 128, :])
                for fo in range(KF):
                    # fp8 PE transpose requires output element step of 2
                    pst = ptp.tile([128, 256], FP8, tag="qtp")
                    pstv = pst[:].rearrange("p (a b) -> p a b", b=2)[:, :, 0]
                    nc.tensor.transpose(pstv,
                                        qstage[:, fo * 128:(fo + 1) * 128],
                                        ident[:])
                    nc.scalar.copy(qT_t[:, fo, tt * 128:(tt + 1) * 128],
                                   pstv)

            # ---- DRAM scratch
            # vproj rows j = w3-projection of v_in row j; chunk-vpad[r] = row
            # r + VOFF; v_in's zero halo rows project to exact zeros
            vproj = dp.tile([TV, F], BF16)
            # apad: 1 guard row + TC data rows + 1 guard row, row = [8 heads x 128]
            apad = dp.tile([TC + 2, H * W], BF16)

            # zero tile for apad guards
            z_t = pers.tile([128, H * W], BF16, tag="zt")
            nc.any.memzero(z_t[:])
            nc.sync.dma_start(apad[0:1, :], z_t[0:1, :])
            nc.sync.dma_start(apad[TC + 1:TC + 2, :], z_t[0:1, :])

            # ---- persistent SBUF activations
            qrT = pers.tile([128, KF, TC], FP8, tag="qrT")   # relu(q @ w1), f-major
            xT = pers.tile([128, KF, TC], BF16, tag="xT")    # band output, f-major

            # ================= Phase A: q-proj + relu (f-major out) ===========
            # PSUM = q @ (16 w1); Relu(psum/16) -> fp8
            for fo in range(KF):
                for tt in range(TC // 512):
                    ps = psm.tile([128, 512], F32, tag="mm")
                    for k in range(KF):
                        nc.tensor.matmul(
                            ps[:], w1_t[:, k, fo * 128:(fo + 1) * 128],
                            qT_t[:, k, tt * 512:(tt + 1) * 512],
                            start=(k == 0), stop=(k == KF - 1))
                    nc.scalar.activation(qrT[:, fo, tt * 512:(tt + 1) * 512], ps[:],
                                         mybir.ActivationFunctionType.Relu,
                                         scale=1.0 / WSCALE)

            # ================= Phase C: v-proj (t-major out) -> vproj =========
            for tb in range(NTV):
                ps = psm.tile([128, 512], F32, tag="mm")
                for k in range(KF):
                    nc.tensor.matmul(
                        ps[:], vT_t[:, k, tb * 128:(tb + 1) * 128],
                        w3_t[:, k, :],
                        start=(k == 0), stop=(k == KF - 1))
                v_sb = wk.tile([128, F], BF16, tag="vsb")
                nc.scalar.copy(v_sb[:], ps[:])
                nc.sync.dma_start(vproj[tb * 128:(tb + 1) * 128, :], v_sb[:])

            # ====== Phase B: s-proj (t-major) + softmax -> apad (padded) ======
            # PSUM = qr @ (16 w2); Exp(psum/16)
            for tb in range(NT128):
                ps = psm.tile([128, 512], F32, tag="mm")
                for k in range(KF):
                    nc.tensor.matmul(
                        ps[:, 0:HC], qrT[:, k, tb * 128:(tb + 1) * 128],
                        w2_t[:, k, :],
                        start=(k == 0), stop=(k == KF - 1))
                e_t = wk.tile([128, HC], F32, tag="et")
                nc.scalar.activation(e_t[:], ps[:, 0:HC],
                                     mybir.ActivationFunctionType.Exp,
                                     scale=1.0 / WSCALE)
                zs = wk.tile([128, H], F32, tag="zs")
                nc.vector.reduce_sum(zs[:], e_t[:].rearrange("p (h c) -> p h c", c=C),
                                     axis=mybir.AxisListType.X)
                rz = wk.tile([128, H], F32, tag="rz")
                nc.vector.reciprocal(rz[:], zs[:])
                ap_t = wk.tile([128, H * W], BF16, tag="apad")
                if tb < 2:
                    # zero the pad region once per pool slot (bufs=2); the pad
                    # columns are never overwritten afterwards
                    nc.any.memzero(ap_t[:])
                nc.vector.tensor_mul(
                    out=ap_t[:].rearrange("p (h w) -> p h w", w=W)[:, :, 0:C],
                    in0=e_t[:].rearrange("p (h c) -> p h c", c=C),
                    in1=rz[:, :, None].to_broadcast((128, H, C)))
                nc.sync.dma_start(apad[1 + tb * 128:1 + (tb + 1) * 128, :], ap_t[:])

            # ================= Phase D: banded attention matmuls ==============
            # x[t', h*64+d] = sum_s chunkvpad[t0+s, h*64+d] * B_h[s, t']
            # B_h loaded via transpose-DMA of sheared apad view.
            apad_h = apad.tensor  # underlying DRAM handle
            apad_off = apad.offset if isinstance(apad.offset, int) else 0
            for g in range(NB // 4):    # groups of 4 band blocks = 256 t'
                pss = [psb.tile([128, 512], F32, tag="px", name=f"px{g}_{pi}")
                       for pi in range(4)]
                for j in range(4):
                    bi = g * 4 + j
                    t0 = S * bi
                    vsp = wk.tile([128, F], BF16, tag="vsp")
                    nc.sync.dma_start(vsp[:], vproj[VOFF + t0:VOFF + t0 + 128, :])
                    for p in range(4):      # head pairs
                        for i in range(2):
                            h = 2 * p + i
                            b_t = bp.tile([W, S], BF16, tag="bt")
                            src = bass.AP(
                                tensor=apad_h,
                                offset=apad_off + (1 + t0) * (H * W) + h * W,
                                ap=[[H * W - 1, S], [1, W]])
                            eng = nc.scalar if h % 2 else nc.sync
                            eng.dma_start_transpose(b_t[:], src)
                            # lhsT = v head-pair [128, 128]; valid out rows are
                            # [i*64:(i+1)*64]; the other half is garbage and
                            # ignored at copyback.
                            nc.tensor.matmul(
                                pss[p][:, j * 128 + i * 64: j * 128 + (i + 1) * 64],
                                vsp[:, p * 128:(p + 1) * 128], b_t[:],
                                start=True, stop=True)
                # copy valid quadrants -> xT (f-major): fold p rows 0:63 = head
                # 2p (cols i=0), rows 64:127 = head 2p+1 (cols i=1)
                for p in range(4):
                    ps3 = pss[p][:].rearrange("d (j i k) -> d j i k", j=4, i=2)
                    dst = xT[:, p, g * 256:(g + 1) * 256] \
                        .rearrange("d (j k) -> d j k", j=4)
                    nc.vector.tensor_copy(out=dst[0:64], in_=ps3[0:64, :, 0, :])
                    nc.vector.tensor_copy(out=dst[64:128], in_=ps3[64:128, :, 1, :])

            # ================= Phase E: out-proj (t-major out) ================
            for tb in range(NT128):
                ps = psm.tile([128, 512], F32, tag="mm")
                for k in range(KF):
                    nc.tensor.matmul(
                        ps[:], xT[:, k, tb * 128:(tb + 1) * 128],
                        wo_t[:, k, :],
                        start=(k == 0), stop=(k == KF - 1))
                o_sb = wk.tile([128, F], BF16, tag="osb")
                nc.scalar.copy(o_sb[:], ps[:])
                nc.sync.dma_start(out[tb * 128:(tb + 1) * 128, :], o_sb[:])

    nc.compile()
    return nc


def _make_exec(nc, devices, donate_input=None):
    """Cached jitted executable + on-device zeros maker for one bass module.

    With donate_input=<name>, that input is donated and XLA aliases its
    buffer as the (shape/dtype-matching) output — no zero buffers needed."""
    import jax
    import jax.numpy as jnp
    from jax.sharding import Mesh, PartitionSpec, NamedSharding
    from jax.experimental.shard_map import shard_map
    from concourse.bass2jax import _bass_exec_p, partition_id_tensor

    partition_name = (nc.partition_id_tensor.name
                      if nc.partition_id_tensor else None)
    in_names, out_names, out_avals = [], [], []
    for alloc in nc.m.functions[0].allocations:
        if not isinstance(alloc, mybir.MemoryLocationSet):
            continue
        if alloc.kind not in ("ExternalInput", "ExternalOutput"):
            continue
        name = alloc.memorylocations[0].name
        if alloc.kind == "ExternalInput":
            if name != partition_name:
                in_names.append(name)
        else:
            out_avals.append(jax.core.ShapedArray(
                tuple(alloc.tensor_shape), mybir.dt.np(alloc.dtype)))
            out_names.append(name)
    n_params, n_outs = len(in_names), len(out_avals)
    in_names_all = list(in_names) + list(out_names)
    if partition_name is not None:
        in_names_all.append(partition_name)

    def _body(*args):
        operands = list(args)
        if partition_name is not None:
            operands.append(partition_id_tensor())
        return tuple(_bass_exec_p.bind(
            *operands,
            out_avals=tuple(out_avals),
            in_names=tuple(in_names_all),
            out_names=tuple(out_names),
            lowering_input_output_aliases=(),
            sim_require_finite=True,
            sim_require_nnan=True,
            nc=nc))

    n = len(devices)
    mesh = Mesh(np.asarray(devices), ("core",))
    shard = NamedSharding(mesh, PartitionSpec("core"))
    if donate_input is None:
        n_args = n_params + n_outs
        donate = tuple(range(n_params, n_args))
        mkzeros = jax.jit(
            lambda: tuple(jnp.zeros((n * a.shape[0], *a.shape[1:]), a.dtype)
                          for a in out_avals),
            out_shardings=(shard,) * n_outs)
        body = _body
    else:
        # outputs alias the donated input's buffer; no zero operands
        n_args = n_params
        donate = (in_names.index(donate_input),)
        mkzeros = None
        in_names_all[:] = list(in_names)
        if partition_name is not None:
            in_names_all.append(partition_name)
        body = _body
    in_specs = (PartitionSpec("core"),) * n_args
    out_specs = (PartitionSpec("core"),) * n_outs
    sharded = jax.jit(
        shard_map(body, mesh=mesh, in_specs=in_specs, out_specs=out_specs,
                  check_rep=False),
        donate_argnums=donate, keep_unused=True)
    return {"sharded": sharded, "mkzeros": mkzeros, "in_names": in_names,
            "out_names": out_names, "shard": shard, "n": n}


def _get_state():
    if "state" in _CACHE:
        return _CACHE["state"]
    import jax
    from concourse.bass2jax import install_neuronx_cc_hook
    install_neuronx_cc_hook()
    devices = jax.devices()[:B]
    wexec = _make_exec(_build_w(), devices)
    kexec = _make_exec(_build_k(), devices, donate_input="v")
    state = {"w": wexec, "k": kexec}
    _CACHE["state"] = state
    return state


def _to_bf16_bits(x32):
    """fp32 -> bf16 via round-half-up on the upper 16 bits (RNE-grade error,
    much faster than ml_dtypes astype). Returns uint16 bit pattern."""
    if x32.strides[-1] != 4:
        x32 = np.ascontiguousarray(x32)
    tmp = x32.view(np.uint32) + np.uint32(0x8000)
    np.right_shift(tmp, 16, out=tmp)
    return tmp.astype(np.uint16)


def kernel(query, key, value, w1, w2, w3, w_out, _trace=False):
    # Host has a single CPU core: interleave each (cheap) conversion with the
    # async uploads so the network stays busy from ~40ms in.
    import jax
    st = _get_state()
    e4 = ml_dtypes.float8_e4m3
    bf = ml_dtypes.bfloat16
    wx, kx = st["w"], st["k"]

    query = np.asarray(query)
    value = np.asarray(value)

    # ---- weights launch first: tiny upload, runs while v/q upload
    wzeros = wx["mkzeros"]()
    warrs = {"w1s": (np.asarray(w1) * WSCALE).astype(e4),
             "w2s": (np.asarray(w2) * WSCALE).astype(e4),
             "w3s": _to_bf16_bits(np.asarray(w3)).view(bf),
             "wos": _to_bf16_bits(np.asarray(w_out)).view(bf)}
    wouts = wx["sharded"](*[warrs[n] for n in wx["in_names"]], *wzeros)
    wfull = dict(zip(wx["out_names"], wouts))

    # ---- per-chunk compute launches, pipelined; convert -> put per tensor
    pending = []
    for ci in range(NCHUNK):
        c0 = ci * TC
        # v main part first (biggest upload; donated -> output buffer)
        vb = _to_bf16_bits(value[:, c0:c0 + TC]).view(bf).reshape(B * TC, F)
        v_dev = jax.device_put(vb, kx["shard"])
        # v halo rows (zero-padded at sequence edges)
        hbuf = np.zeros((B, 2 * VH, F), np.uint16)
        if c0 > 0:
            hbuf[:, 0:VH] = _to_bf16_bits(value[:, c0 - VH:c0])
        if c0 + TC < T:
            hbuf[:, VH:2 * VH] = _to_bf16_bits(value[:, c0 + TC:c0 + TC + VH])
        vh_dev = jax.device_put(hbuf.view(bf).reshape(B * 2 * VH, F),
                                kx["shard"])
        # q chunk, t-major fp8 (transposed on device)
        q8 = query[:, c0:c0 + TC, :].astype(e4).reshape(B * TC, F)
        q_dev = jax.device_put(q8, kx["shard"])
        arrays = {"q": q_dev, "v": v_dev, "vh": vh_dev, "w1f": wfull["w1f"],
                  "w2f": wfull["w2f"], "w3f": wfull["w3f"],
                  "wof": wfull["wof"]}
        ins = [arrays[n] for n in kx["in_names"]]
        pending.append(kx["sharded"](*ins))

    # ---- collect: widen bf16 -> fp32 exactly (zero-extension)
    buf = np.zeros((B, T, F, 2), np.uint16)
    for ci, outs in enumerate(pending):
        c0 = ci * TC
        o16 = np.asarray(outs[0]).view(np.uint16).reshape(B, TC, F)
        buf[:, c0:c0 + TC, :, 1] = o16
    return buf.view(np.float32)[..., 0]


# revision 27
# speedup vs baseline: 1.5271x; 1.2318x over previous
"""Trainium2 Bass kernel for LocalDenseSynthesizerAttention.

Data-parallel over batch B=8 -> 8 cores, one batch each. Wire-traffic and
dispatch optimized for the axon tunnel (~90MB/s each way, full duplex):
  - jitted executables built once and cached (no per-call retrace)
  - q shipped t-major fp8 (e4m3) and transposed on-device (PE transpose);
    v shipped t-major bf16 and transposed on-device via XBAR DMA;
    output returned bf16 and widened exactly on host
  - w1/w2 shipped fp8 scaled x16 (rescaled on device via activation scale),
    w3/w_out bf16; shipped as 8-way shards once per call to a tiny
    weights launch that AllGathers them on device; the full per-core
    weights stay device-resident and feed the compute launches
  - compute is split into sequence chunks (the attention window is local,
    halo = 22), one 8-core launch per chunk: chunk i+1's upload overlaps
    chunk i's exec + download
  - donated output buffers created on-device (no zeros upload)

The local window C=45 weighted sum is computed as banded matmuls: the banded
matrix B[s,t'] = attn[t0+t',h,s-t'] is an affine strided view of a zero-padded
attn tensor in DRAM, loaded matmul-ready via XBAR transpose-DMA.

Self-contained: hardcodes shapes from the problem spec.
"""
import sys
sys.path.insert(0, '/opt/trn_rl_repo')
import numpy as np
import ml_dtypes

import concourse.bass as bass
import concourse.mybir as mybir
import concourse.tile as tile
from concourse import bacc
from concourse import masks

T, F = 2048, 512
H, C, DK = 8, 45, 64
HC = H * C          # 360
W = 128             # padded attn width per head (covers s-t' in [-63,127])
S = 64              # t' band-block size
PADV = 22           # (C-1)//2
KF = F // 128       # 4 contraction chunks
B = 8               # total batches / cores
FSH = F // B        # 64 weight-shard rows per core

NCHUNK = 2
TC = T // NCHUNK    # sequence-chunk length
VH = 64             # v halo rows each side (>= PADV, keeps tiles 128-aligned)
TV = TC + 2 * VH    # logical v rows per chunk (main + halo)
VOFF = VH - PADV    # chunk-vpad[r] = v_logical[r + VOFF]

BF16 = mybir.dt.bfloat16
FP8 = mybir.dt.float8e4
F32 = mybir.dt.float32
WSCALE = 16.0       # fp8 weight pre-scale for w1/w2

_CACHE = {}


def _build_w():
    """Tiny weights launch: AllGather 8-way weight shards into full
    per-core weights (device-resident outputs)."""
    nc = bacc.Bacc("TRN2", target_bir_lowering=False, debug=False,
                   num_devices=B)
    w1s = nc.dram_tensor("w1s", (FSH, F), FP8, kind="ExternalInput")
    w2s = nc.dram_tensor("w2s", (FSH, HC), FP8, kind="ExternalInput")
    w3s = nc.dram_tensor("w3s", (FSH, F), BF16, kind="ExternalInput")
    wos = nc.dram_tensor("wos", (FSH, F), BF16, kind="ExternalInput")
    w1f = nc.dram_tensor("w1f", (F, F), FP8, kind="ExternalOutput")
    w2f = nc.dram_tensor("w2f", (F, HC), FP8, kind="ExternalOutput")
    w3f = nc.dram_tensor("w3f", (F, F), BF16, kind="ExternalOutput")
    wof = nc.dram_tensor("wof", (F, F), BF16, kind="ExternalOutput")
    groups = [list(range(B))]
    with tile.TileContext(nc) as tc:
        with tc.tile_pool(name="dram", bufs=1, space="DRAM") as dp:
            # collectives cannot read IO tensors: stage shards first
            stages = (dp.tile([FSH, F], FP8, name="st1"),
                      dp.tile([FSH, HC], FP8, name="st2"),
                      dp.tile([FSH, F], BF16, name="st3"),
                      dp.tile([FSH, F], BF16, name="st4"))
            fulls = (dp.tile([F, F], FP8, name="fu1"),
                     dp.tile([F, HC], FP8, name="fu2"),
                     dp.tile([F, F], BF16, name="fu3"),
                     dp.tile([F, F], BF16, name="fu4"))
            for shard, stage, full, out in zip(
                    (w1s, w2s, w3s, wos), stages, fulls,
                    (w1f, w2f, w3f, wof)):
                nc.sync.dma_start(stage[:, :], shard[:, :])
                nc.gpsimd.collective_compute(
                    "AllGather", mybir.AluOpType.bypass, groups,
                    [stage[:, :]], [full[:, :]])
                nc.sync.dma_start(out[:, :], full[:, :])
    nc.compile()
    return nc


def _build_k():
    """Compute launch for one sequence chunk of TC rows."""
    NT128 = TC // 128           # t-tiles in the chunk
    NTV = TV // 128             # t-tiles of the v input (incl halo)
    NB = TC // S                # band blocks
    nc = bacc.Bacc("TRN2", target_bir_lowering=False, debug=False,
                   num_devices=B)
    q = nc.dram_tensor("q", (TC, F), FP8, kind="ExternalInput")
    # v is split so the main part exactly matches the output shape/dtype and
    # can be donated/aliased as the output buffer (saves a zeros launch):
    # vh rows [0, VH) = rows just before the chunk, [VH, 2VH) = just after
    v = nc.dram_tensor("v", (TC, F), BF16, kind="ExternalInput")
    vh = nc.dram_tensor("vh", (2 * VH, F), BF16, kind="ExternalInput")
    w1f = nc.dram_tensor("w1f", (F, F), FP8, kind="ExternalInput")
    w2f = nc.dram_tensor("w2f", (F, HC), FP8, kind="ExternalInput")
    w3f = nc.dram_tensor("w3f", (F, F), BF16, kind="ExternalInput")
    wof = nc.dram_tensor("wof", (F, F), BF16, kind="ExternalInput")
    out = nc.dram_tensor("out", (TC, F), BF16, kind="ExternalOutput")

    with tile.TileContext(nc) as tc:
        with tc.tile_pool(name="wpool", bufs=1) as wp, \
             tc.tile_pool(name="inpool", bufs=1) as inp, \
             tc.tile_pool(name="persist", bufs=1) as pers, \
             tc.tile_pool(name="work", bufs=2) as wk, \
             tc.tile_pool(name="band", bufs=4) as bp, \
             tc.tile_pool(name="psmain", bufs=2, space="PSUM") as psm, \
             tc.tile_pool(name="psband", bufs=4, space="PSUM") as psb, \
             tc.tile_pool(name="pstp", bufs=2, space="PSUM") as ptp, \
             tc.tile_pool(name="drampool", bufs=1, space="DRAM") as dp:

            # ---- weights to SBUF, [128, KF, n] layout (partition = contraction)
            w1_t = wp.tile([128, KF, F], FP8, tag="w1")
            nc.sync.dma_start(w1_t[:], w1f[:, :].rearrange("(ko p) n -> p ko n", p=128))
            w2_t = wp.tile([128, KF, HC], FP8, tag="w2")
            nc.sync.dma_start(w2_t[:], w2f[:, :].rearrange("(ko p) n -> p ko n", p=128))
            w3_t = wp.tile([128, KF, F], BF16, tag="w3")
            nc.sync.dma_start(w3_t[:], w3f[:, :].rearrange("(ko p) n -> p ko n", p=128))
            wo_t = wp.tile([128, KF, F], BF16, tag="wo")
            nc.sync.dma_start(wo_t[:], wof[:, :].rearrange("(ko p) n -> p ko n", p=128))

            # ---- v (t-major bf16): XBAR transpose to f-major
            # vT_t cols: [0, VH) front halo | [VH, VH+TC) main | back halo
            vT_t = inp.tile([128, KF, TV], BF16, tag="vT")
            vhT = inp.tile([128, KF, 2 * VH], BF16, tag="vhT")
            for fo in range(KF):
                eng = nc.scalar if fo % 2 else nc.sync
                eng.dma_start_transpose(vT_t[:, fo, VH:VH + TC],
                                        v[:, fo * 128:(fo + 1) * 128])
                eng.dma_start_transpose(vhT[:, fo, :],
                                        vh[:, fo * 128:(fo + 1) * 128])
            nc.vector.tensor_copy(out=vT_t[:, :, 0:VH], in_=vhT[:, :, 0:VH])
            nc.vector.tensor_copy(out=vT_t[:, :, VH + TC:TV],
                                  in_=vhT[:, :, VH:2 * VH])

            # ---- q (t-major fp8): PE-transpose to f-major
            ident = pers.tile([128, 128], FP8, tag="ident")
            masks.make_identity(nc, ident[:])
            qT_t = inp.tile([128, KF, TC], FP8, tag="qT")
            for tt in range(NT128):
                qstage = wk.tile([128, F], FP8, tag="qstage")
                nc.sync.dma_start(qstage[:], q[tt * 128:(tt + 1) * 128, :])
                for fo in range(KF):
                    # fp8 PE transpose requires output element step of 2
                    pst = ptp.tile([128, 256], FP8, tag="qtp")
                    pstv = pst[:].rearrange("p (a b) -> p a b", b=2)[:, :, 0]
                    nc.tensor.transpose(pstv,
                                        qstage[:, fo * 128:(fo + 1) * 128],
                                        ident[:])
                    nc.scalar.copy(qT_t[:, fo, tt * 128:(tt + 1) * 128],
                                   pstv)

            # ---- DRAM scratch
            # vproj rows j = w3-projection of v_in row j; chunk-vpad[r] = row
            # r + VOFF; v_in's zero halo rows project to exact zeros
            vproj = dp.tile([TV, F], BF16)
            # apad: 1 guard row + TC data rows + 1 guard row, row = [8 heads x 128]
            apad = dp.tile([TC + 2, H * W], BF16)

            # zero tile for apad guards
            z_t = pers.tile([128, H * W], BF16, tag="zt")
            nc.any.memzero(z_t[:])
            nc.sync.dma_start(apad[0:1, :], z_t[0:1, :])
            nc.sync.dma_start(apad[TC + 1:TC + 2, :], z_t[0:1, :])

            # ---- persistent SBUF activations
            qrT = pers.tile([128, KF, TC], FP8, tag="qrT")   # relu(q @ w1), f-major
            xT = pers.tile([128, KF, TC], BF16, tag="xT")    # band output, f-major

            # ================= Phase A: q-proj + relu (f-major out) ===========
            # PSUM = q @ (16 w1); Relu(psum/16) -> fp8
            for fo in range(KF):
                for tt in range(TC // 512):
                    ps = psm.tile([128, 512], F32, tag="mm")
                    for k in range(KF):
                        nc.tensor.matmul(
                            ps[:], w1_t[:, k, fo * 128:(fo + 1) * 128],
                            qT_t[:, k, tt * 512:(tt + 1) * 512],
                            start=(k == 0), stop=(k == KF - 1))
                    nc.scalar.activation(qrT[:, fo, tt * 512:(tt + 1) * 512], ps[:],
                                         mybir.ActivationFunctionType.Relu,
                                         scale=1.0 / WSCALE)

            # ================= Phase C: v-proj (t-major out) -> vproj =========
            for tb in range(NTV):
                ps = psm.tile([128, 512], F32, tag="mm")
                for k in range(KF):
                    nc.tensor.matmul(
                        ps[:], vT_t[:, k, tb * 128:(tb + 1) * 128],
                        w3_t[:, k, :],
                        start=(k == 0), stop=(k == KF - 1))
                v_sb = wk.tile([128, F], BF16, tag="vsb")
                nc.scalar.copy(v_sb[:], ps[:])
                nc.sync.dma_start(vproj[tb * 128:(tb + 1) * 128, :], v_sb[:])

            # ====== Phase B: s-proj (t-major) + softmax -> apad (padded) ======
            # PSUM = qr @ (16 w2); Exp(psum/16)
            for tb in range(NT128):
                ps = psm.tile([128, 512], F32, tag="mm")
                for k in range(KF):
                    nc.tensor.matmul(
                        ps[:, 0:HC], qrT[:, k, tb * 128:(tb + 1) * 128],
                        w2_t[:, k, :],
                        start=(k == 0), stop=(k == KF - 1))
                e_t = wk.tile([128, HC], F32, tag="et")
                nc.scalar.activation(e_t[:], ps[:, 0:HC],
                                     mybir.ActivationFunctionType.Exp,
                                     scale=1.0 / WSCALE)
                zs = wk.tile([128, H], F32, tag="zs")
                nc.vector.reduce_sum(zs[:], e_t[:].rearrange("p (h c) -> p h c", c=C),
                                     axis=mybir.AxisListType.X)
                rz = wk.tile([128, H], F32, tag="rz")
                nc.vector.reciprocal(rz[:], zs[:])
                ap_t = wk.tile([128, H * W], BF16, tag="apad")
                if tb < 2:
                    # zero the pad region once per pool slot (bufs=2); the pad
                    # columns are never overwritten afterwards
                    nc.any.memzero(ap_t[:])
                nc.vector.tensor_mul(
                    out=ap_t[:].rearrange("p (h w) -> p h w", w=W)[:, :, 0:C],
                    in0=e_t[:].rearrange("p (h c) -> p h c", c=C),
                    in1=rz[:, :, None].to_broadcast((128, H, C)))
                nc.sync.dma_start(apad[1 + tb * 128:1 + (tb + 1) * 128, :], ap_t[:])

            # ================= Phase D: banded attention matmuls ==============
            # x[t', h*64+d] = sum_s chunkvpad[t0+s, h*64+d] * B_h[s, t']
            # B_h loaded via transpose-DMA of sheared apad view.
            apad_h = apad.tensor  # underlying DRAM handle
            apad_off = apad.offset if isinstance(apad.offset, int) else 0
            for g in range(NB // 4):    # groups of 4 band blocks = 256 t'
                pss = [psb.tile([128, 512], F32, tag="px", name=f"px{g}_{pi}")
                       for pi in range(4)]
                for j in range(4):
                    bi = g * 4 + j
                    t0 = S * bi
                    vsp = wk.tile([128, F], BF16, tag="vsp")
                    nc.sync.dma_start(vsp[:], vproj[VOFF + t0:VOFF + t0 + 128, :])
                    for p in range(4):      # head pairs
                        for i in range(2):
                            h = 2 * p + i
                            b_t = bp.tile([W, S], BF16, tag="bt")
                            src = bass.AP(
                                tensor=apad_h,
                                offset=apad_off + (1 + t0) * (H * W) + h * W,
                                ap=[[H * W - 1, S], [1, W]])
                            eng = nc.scalar if h % 2 else nc.sync
                            eng.dma_start_transpose(b_t[:], src)
                            # lhsT = v head-pair [128, 128]; valid out rows are
                            # [i*64:(i+1)*64]; the other half is garbage and
                            # ignored at copyback.
                            nc.tensor.matmul(
                                pss[p][:, j * 128 + i * 64: j * 128 + (i + 1) * 64],
                                vsp[:, p * 128:(p + 1) * 128], b_t[:],
                                start=True, stop=True)
                # copy valid quadrants -> xT (f-major): fold p rows 0:63 = head
                # 2p (cols i=0), rows 64:127 = head 2p+1 (cols i=1)
                for p in range(4):
                    ps3 = pss[p][:].rearrange("d (j i k) -> d j i k", j=4, i=2)
                    dst = xT[:, p, g * 256:(g + 1) * 256] \
                        .rearrange("d (j k) -> d j k", j=4)
                    nc.vector.tensor_copy(out=dst[0:64], in_=ps3[0:64, :, 0, :])
                    nc.vector.tensor_copy(out=dst[64:128], in_=ps3[64:128, :, 1, :])

            # ================= Phase E: out-proj (t-major out) ================
            for tb in range(NT128):
                ps = psm.tile([128, 512], F32, tag="mm")
                for k in range(KF):
                    nc.tensor.matmul(
                        ps[:], xT[:, k, tb * 128:(tb + 1) * 128],
                        wo_t[:, k, :],
                        start=(k == 0), stop=(k == KF - 1))
                o_sb = wk.tile([128, F], BF16, tag="osb")
                nc.scalar.copy(o_sb[:], ps[:])
                nc.sync.dma_start(out[tb * 128:(tb + 1) * 128, :], o_sb[:])

    nc.compile()
    return nc


def _make_exec(nc, devices, donate_input=None):
    """Cached jitted executable + on-device zeros maker for one bass module.

    With donate_input=<name>, that input is donated and XLA aliases its
    buffer as the (shape/dtype-matching) output — no zero buffers needed."""
    import jax
    import jax.numpy as jnp
    from jax.sharding import Mesh, PartitionSpec, NamedSharding
    from jax.experimental.shard_map import shard_map
    from concourse.bass2jax import _bass_exec_p, partition_id_tensor

    partition_name = (nc.partition_id_tensor.name
                      if nc.partition_id_tensor else None)
    in_names, out_names, out_avals = [], [], []
    for alloc in nc.m.functions[0].allocations:
        if not isinstance(alloc, mybir.MemoryLocationSet):
            continue
        if alloc.kind not in ("ExternalInput", "ExternalOutput"):
            continue
        name = alloc.memorylocations[0].name
        if alloc.kind == "ExternalInput":
            if name != partition_name:
                in_names.append(name)
        else:
            out_avals.append(jax.core.ShapedArray(
                tuple(alloc.tensor_shape), mybir.dt.np(alloc.dtype)))
            out_names.append(name)
    n_params, n_outs = len(in_names), len(out_avals)
    in_names_all = list(in_names) + list(out_names)
    if partition_name is not None:
        in_names_all.append(partition_name)

    def _body(*args):
        operands = list(args)
        if partition_name is not None:
            operands.append(partition_id_tensor())
        return tuple(_bass_exec_p.bind(
            *operands,
            out_avals=tuple(out_avals),
            in_names=tuple(in_names_all),
            out_names=tuple(out_names),
            lowering_input_output_aliases=(),
            sim_require_finite=True,
            sim_require_nnan=True,
            nc=nc))

    n = len(devices)
    mesh = Mesh(np.asarray(devices), ("core",))
    shard = NamedSharding(mesh, PartitionSpec("core"))
    if donate_input is None:
        n_args = n_params + n_outs
        donate = tuple(range(n_params, n_args))
        mkzeros = jax.jit(
            lambda: tuple(jnp.zeros((n * a.shape[0], *a.shape[1:]), a.dtype)
                          for a in out_avals),
            out_shardings=(shard,) * n_outs)
        body = _body
    else:
        # outputs alias the donated input's buffer; no zero operands
        n_args = n_params
        donate = (in_names.index(donate_input),)
        mkzeros = None
        in_names_all[:] = list(in_names)
        if partition_name is not None:
            in_names_all.append(partition_name)
        body = _body
    in_specs = (PartitionSpec("core"),) * n_args
    out_specs = (PartitionSpec("core"),) * n_outs
    sharded = jax.jit(
        shard_map(body, mesh=mesh, in_specs=in_specs, out_specs=out_specs,
                  check_rep=False),
        donate_argnums=donate, keep_unused=True)
    return {"sharded": sharded, "mkzeros": mkzeros, "in_names": in_names,
            "out_names": out_names, "shard": shard, "n": n}


def _get_state():
    if "state" in _CACHE:
        return _CACHE["state"]
    import jax
    from concourse.bass2jax import install_neuronx_cc_hook
    install_neuronx_cc_hook()
    devices = jax.devices()[:B]
    wexec = _make_exec(_build_w(), devices)
    kexec = _make_exec(_build_k(), devices, donate_input="v")
    state = {"w": wexec, "k": kexec}
    _CACHE["state"] = state
    return state


def _to_bf16_bits(x32):
    """fp32 -> bf16 via round-half-up on the upper 16 bits (RNE-grade error,
    much faster than ml_dtypes astype). Returns uint16 bit pattern."""
    if x32.strides[-1] != 4:
        x32 = np.ascontiguousarray(x32)
    tmp = x32.view(np.uint32) + np.uint32(0x8000)
    np.right_shift(tmp, 16, out=tmp)
    return tmp.astype(np.uint16)


def kernel(query, key, value, w1, w2, w3, w_out, _trace=False):
    # Host has a single CPU core: interleave each (cheap) conversion with the
    # async uploads so the network stays busy from ~40ms in.
    import jax
    st = _get_state()
    e4 = ml_dtypes.float8_e4m3
    bf = ml_dtypes.bfloat16
    wx, kx = st["w"], st["k"]

    query = np.asarray(query)
    value = np.asarray(value)

    # ---- weights launch first: tiny upload, runs while v/q upload
    wzeros = wx["mkzeros"]()
    warrs = {"w1s": (np.asarray(w1) * WSCALE).astype(e4),
             "w2s": (np.asarray(w2) * WSCALE).astype(e4),
             "w3s": _to_bf16_bits(np.asarray(w3)).view(bf),
             "wos": _to_bf16_bits(np.asarray(w_out)).view(bf)}
    wouts = wx["sharded"](*[warrs[n] for n in wx["in_names"]], *wzeros)
    wfull = dict(zip(wx["out_names"], wouts))

    # ---- per-chunk compute launches, pipelined; convert -> put per tensor
    pending = []
    for ci in range(NCHUNK):
        c0 = ci * TC
        # v main part first (biggest upload; donated -> output buffer)
        vb = _to_bf16_bits(value[:, c0:c0 + TC]).view(bf).reshape(B * TC, F)
        v_dev = jax.device_put(vb, kx["shard"])
        # v halo rows (zero-padded at sequence edges)
        hbuf = np.zeros((B, 2 * VH, F), np.uint16)
        if c0 > 0:
            hbuf[:, 0:VH] = _to_bf16_bits(value[:, c0 - VH:c0])
        if c0 + TC < T:
            hbuf[:, VH:2 * VH] = _to_bf16_bits(value[:, c0 + TC:c0 + TC + VH])
        vh_dev = jax.device_put(hbuf.view(bf).reshape(B * 2 * VH, F),
                                kx["shard"])
        # q chunk, t-major fp8 (transposed on device)
        q8 = query[:, c0:c0 + TC, :].astype(e4).reshape(B * TC, F)
        q_dev = jax.device_put(q8, kx["shard"])
        arrays = {"q": q_dev, "v": v_dev, "vh": vh_dev, "w1f": wfull["w1f"],
                  "w2f": wfull["w2f"], "w3f": wfull["w3f"],
                  "wof": wfull["wof"]}
        ins = [arrays[n] for n in kx["in_names"]]
        outs = kx["sharded"](*ins)
        outs[0].copy_to_host_async()   # start D2H as soon as exec finishes
        pending.append(outs)

    # ---- collect: widen bf16 -> fp32 exactly (zero-extension)
    buf = np.zeros((B, T, F, 2), np.uint16)
    for ci, outs in enumerate(pending):
        c0 = ci * TC
        o16 = np.asarray(outs[0]).view(np.uint16).reshape(B, TC, F)
        buf[:, c0:c0 + TC, :, 1] = o16
    return buf.view(np.float32)[..., 0]
